# revision 1
# baseline (speedup 1.0000x reference)
"""Trainium2 Bass kernel: GQA causal attention (B=2, S=2048, H=2048, 16 q-heads,
4 kv-heads, head_dim=128), tensor-parallel over 8 NeuronCores.

Sharding: 2 q-heads + their (shared) kv-head per core; wq/wk/wv column-sharded,
wo row-sharded.  Each core computes a partial o_proj output; the host sums the
8 partials (the standard TP partial-sum unshard).

All on-chip layouts are transposed (feature-on-partition) so every matmul
contraction runs over the partition axis with N=512-wide moving operands
(float32r at full PE rate):
  qkvT  = w.T @ x.T                       (PE, accumulate over 16 h-chunks)
  RoPE:   q' = q*cos + (M@q)*sin          (rot via tiny PE matmul, combine on DVE)
  scoresT[k,q] = K @ Q^T                  (PE; wq pre-scaled by 1/sqrt(D))
  P^T   = exp(scoresT - 40)               (ACT, fused scale/bias; exact softmax
                                           after normalization: const cancels)
  causal mask: affine_select on P^T       (GpSimd, fill=0)
  outT  = V^T @ P^T ; rowsum = 1^T @ P^T  (PE, PSUM-accumulated over k-chunks)
  outT *= bcast(1/rowsum)                 (PE outer-product bcast + DVE recip/mul)
  out_partial = outT.T @ wo_c             (PE, outT is already the needed lhsT)
"""

import os
import sys
import time

import numpy as np

sys.path.insert(0, "/opt/trn_rl_repo")

from contextlib import ExitStack

import concourse.bass as bass
from concourse import bacc
import concourse.mybir as mybir
import concourse.tile as tile
from concourse.bass_utils import run_bass_kernel_spmd
from concourse.masks import make_identity

F32 = mybir.dt.float32
F32R = mybir.dt.float32r
AF = mybir.ActivationFunctionType

B, S, H = 2, 2048, 2048
NH, KVH, D = 16, 4, 128
NCORES = 8
HPC = NH // NCORES  # q heads per core = 2
R = B * S  # 4096 flattened rows
QKV_W = HPC * D + 2 * D  # 512 = [q0|q1|k|v] columns per core
NB_RB = R // 512  # 8 row-blocks of 512
NB_HC = H // 128  # 16 contraction chunks
SB = S // 512  # 4 q-blocks per batch
SC = S // 128  # 16 k-chunks per batch
EXP_BIAS = -40.0

LAST_EXEC_TIME_NS = None
LAST_RESULTS = None


def build_graph(reps=1):
    nc = bacc.Bacc(
        "TRN2", target_bir_lowering=False, debug=False, num_devices=NCORES
    )
    xT = nc.dram_tensor("xT", [H, R], F32R, kind="ExternalInput").ap()
    wqkv = nc.dram_tensor("wqkv", [H, QKV_W], F32R, kind="ExternalInput").ap()
    wo = nc.dram_tensor("wo", [HPC * D, H], F32R, kind="ExternalInput").ap()
    cosT = nc.dram_tensor("cosT", [D, S], F32, kind="ExternalInput").ap()
    sinT = nc.dram_tensor("sinT", [D, S], F32, kind="ExternalInput").ap()
    rotMT = nc.dram_tensor("rotMT", [D, D], F32R, kind="ExternalInput").ap()
    onesd = nc.dram_tensor("onesd", [D, D], F32R, kind="ExternalInput").ap()
    out = nc.dram_tensor("out", [R, H], F32, kind="ExternalOutput").ap()

    with tile.TileContext(nc) as tc, ExitStack() as ctx:
        # ---- persistent SBUF ----
        const_pool = ctx.enter_context(tc.tile_pool(name="const", bufs=1))
        w_sb = const_pool.tile([128, NB_HC, QKV_W], F32R)
        wo_sb = const_pool.tile([128, HPC, H], F32R)
        cos_sb = const_pool.tile([128, S], F32)
        sin_sb = const_pool.tile([128, S], F32)
        rot_sb = const_pool.tile([128, D], F32R)
        ident_sb = const_pool.tile([128, 128], F32)
        ones_sb = const_pool.tile([128, 128], F32R)  # all-ones (rowsum/bcast lhsT)
        expb_sb = const_pool.tile([128, 1], F32)  # exp bias (per-partition)
        qk_sb = const_pool.tile([128, 3, R], F32R)  # roped q0T|q1T|kT, all rows
        v_sb = const_pool.tile([128, R // 128, D], F32R)  # V natural, s-chunked

        make_identity(nc, ident_sb[:])
        nc.gpsimd.memset(expb_sb[:], EXP_BIAS)

        for _rep in range(reps):
            # ---- phase 1: qkvT matmuls + RoPE + V transpose ----
            # Weight chunks stream interleaved with rb0's x-tiles so the first
            # matmul's deps land after ~512KB of DMA, not after all constants.
            with (
                tc.tile_pool(name="xt", bufs=8) as xt_pool,
                tc.tile_pool(name="qkv_ps", bufs=5, space="PSUM") as qkv_ps_pool,
                tc.tile_pool(name="rot_ps", bufs=1, space="PSUM") as rot_ps_pool,
                tc.tile_pool(name="vt_ps", bufs=2, space="PSUM") as vt_ps_pool,
                tc.tile_pool(name="qraw", bufs=3) as qraw_pool,
                tc.tile_pool(name="rtmp", bufs=3) as rtmp_pool,
            ):
                for rb in range(NB_RB):
                    ps = [
                        qkv_ps_pool.tile([128, 512], F32, tag="qkvps", name=f"qkvps{cg}")
                        for cg in range(4)
                    ]
                    for hc in range(NB_HC):
                        if rb == 0:
                            nc.sync.dma_start(
                                w_sb[:, hc, :], wqkv[hc * 128 : (hc + 1) * 128, :]
                            )
                        xt = xt_pool.tile([128, 512], F32R)
                        nc.sync.dma_start(
                            xt[:], xT[hc * 128 : (hc + 1) * 128, rb * 512 : (rb + 1) * 512]
                        )
                        if rb == 0 and hc == 2:
                            nc.sync.dma_start(rot_sb[:], rotMT)
                        if rb < SB and hc == 8:
                            sl = slice(rb * 512, (rb + 1) * 512)
                            nc.sync.dma_start(cos_sb[:, sl], cosT[:, sl])
                        if rb < SB and hc == 11:
                            sl = slice(rb * 512, (rb + 1) * 512)
                            nc.sync.dma_start(sin_sb[:, sl], sinT[:, sl])
                        if rb == 1 and hc == 0:
                            nc.sync.dma_start(ones_sb[:], onesd)
                        if rb == 4 and hc == 0:
                            for h in range(HPC):
                                nc.sync.dma_start(
                                    wo_sb[:, h, :], wo[h * 128 : (h + 1) * 128, :]
                                )
                        for cg in range(4):
                            nc.tensor.matmul(
                                ps[cg][:],
                                w_sb[:, hc, cg * 128 : (cg + 1) * 128],
                                xt[:],
                                start=(hc == 0),
                                stop=(hc == NB_HC - 1),
                            )
                    cs = cos_sb[:, (rb % SB) * 512 : (rb % SB + 1) * 512]
                    sn = sin_sb[:, (rb % SB) * 512 : (rb % SB + 1) * 512]
                    for cg in range(3):  # q0, q1, k get RoPE
                        qraw = qraw_pool.tile([128, 512], F32R)
                        nc.scalar.copy(qraw[:], ps[cg][:])
                        rps = rot_ps_pool.tile([128, 512], F32)
                        nc.tensor.matmul(rps[:], rot_sb[:], qraw[:])
                        m1 = rtmp_pool.tile([128, 512], F32, tag="rtmp")
                        nc.vector.tensor_mul(m1[:], ps[cg][:], cs)
                        m2 = rtmp_pool.tile([128, 512], F32, tag="rtmp")
                        nc.vector.tensor_mul(m2[:], rps[:], sn)
                        nc.vector.tensor_add(
                            qk_sb[:, cg, rb * 512 : (rb + 1) * 512], m1[:], m2[:]
                        )
                    # v: copy psum -> sbuf, PE-transpose 128x128 tiles into v_sb
                    vtmp = qraw_pool.tile([128, 512], F32, tag="qraw")
                    nc.vector.tensor_copy(vtmp[:], ps[3][:])
                    for t in range(4):
                        vt_ps = vt_ps_pool.tile([128, 128], F32)
                        nc.tensor.transpose(
                            vt_ps[:], vtmp[:, t * 128 : (t + 1) * 128], ident_sb[:]
                        )
                        nc.vector.tensor_copy(v_sb[:, rb * 4 + t, :], vt_ps[:])

            # ---- phase 2: attention with o_proj drip-fed between groups ----
            # o_proj tiles are emitted one-per-attention-group from a pending
            # FIFO, so the in-order PE queue always has attention matmuls between
            # an o_proj pair and its psum-slot dependency (the psum->sbuf copy).
            with (
                tc.tile_pool(name="outT", bufs=2) as outT_pool,
                tc.tile_pool(name="st_ps", bufs=2, space="PSUM") as st_ps_pool,
                tc.tile_pool(name="ot_ps", bufs=2, space="PSUM") as ot_ps_pool,
                tc.tile_pool(name="rs_ps", bufs=2, space="PSUM") as rs_ps_pool,
                tc.tile_pool(name="ms_ps", bufs=2, space="PSUM") as ms_ps_pool,
                tc.tile_pool(name="pt", bufs=12) as pt_pool,
                tc.tile_pool(name="s2", bufs=2) as s2_pool,
                tc.tile_pool(name="rr_sb", bufs=2) as rr_sb_pool,
                tc.tile_pool(name="osb", bufs=7) as osb_pool,
            ):
                pending = []
                ncopy = [0]

                def emit_op(nmax, split=False):
                    for _ in range(nmax):
                        if not pending:
                            return
                        ob, oT, st, nb = pending.pop(0)
                        op_ps = ms_ps_pool.tile(
                            [128, 512], F32, tag="ms", name="op_ps"
                        )
                        for h in range(HPC):
                            nc.tensor.matmul(
                                op_ps[:],
                                oT[:, h, st * 128 : (st + 1) * 128],
                                wo_sb[:, h, nb * 512 : (nb + 1) * 512],
                                start=(h == 0),
                                stop=(h == HPC - 1),
                            )
                        osb = osb_pool.tile([128, 512], F32)
                        if split:  # tail flush: halve latency on both engines
                            nc.vector.tensor_copy(osb[:, 0:256], op_ps[:, 0:256])
                            nc.scalar.copy(osb[:, 256:512], op_ps[:, 256:512])
                        elif ncopy[0] % 2 == 0:
                            nc.vector.tensor_copy(osb[:], op_ps[:])
                        else:
                            nc.scalar.copy(osb[:], op_ps[:])
                        ncopy[0] += 1
                        r0 = ob * S + st * 128
                        nc.sync.dma_start(
                            out[r0 : r0 + 128, nb * 512 : (nb + 1) * 512], osb[:]
                        )

                PD = 5  # per-head pipeline depth (2 heads => 2*PD global)
                for b in range(B):
                    outT = outT_pool.tile([128, HPC, S], F32R)
                    kTh = qk_sb[:, 2, b * S : (b + 1) * S]
                    for qb in range(SB):
                        o_ps, r_ps = {}, {}
                        for h in range(HPC):
                            o_ps[h] = ot_ps_pool.tile(
                                [128, 512], F32, tag="ot", name=f"ot{h}"
                            )
                            r_ps[h] = rs_ps_pool.tile(
                                [128, 512], F32, tag="rs", name=f"rs{h}"
                            )
                        nj = 4 * qb + 4
                        pts = {}
                        s2s = {}
                        first_rs = 1 if qb > 0 else 0
                        for jj in range(nj + PD):
                            emit_op(1)
                            if jj < nj:
                                j = jj
                                # diagonal band: only q >= k is needed; narrow
                                # the moving operand accordingly.
                                r = j - 4 * qb
                                # fp32r needs N>=256 for full rate: cap the
                                # narrowing at 256 (r=3 keeps W=256, the mask
                                # zeroes the extra quarter).
                                qoff = min(128 * r, 256) if r > 0 else 0
                                W = 512 - qoff
                                for h in range(HPC):
                                    qTh = qk_sb[:, h, b * S : (b + 1) * S]
                                    s_ps = st_ps_pool.tile([128, 512], F32)
                                    nc.tensor.matmul(
                                        s_ps[:, 0:W],
                                        kTh[:, j * 128 : (j + 1) * 128],
                                        qTh[:, qb * 512 + qoff : (qb + 1) * 512],
                                        start=True,
                                        stop=True,
                                    )
                                    pt = pt_pool.tile([128, 512], F32R)
                                    nc.scalar.activation(
                                        pt[:, 0:W],
                                        s_ps[:, 0:W],
                                        AF.Exp,
                                        bias=expb_sb[:],
                                        scale=1.0,
                                    )
                                    if r >= 0:  # zero where k > q in this tile
                                        # the mask ramp starts mo cols in; the
                                        # prefix [0:mo) is fully masked.
                                        mo = 128 * r - qoff
                                        if mo > 0:
                                            nc.vector.tensor_scalar_mul(
                                                pt[:, 0:mo], pt[:, 0:mo], 0.0
                                            )
                                        # the ramp qq' >= kk only bites in
                                        # the first 128 cols after the shift
                                        nc.gpsimd.affine_select(
                                            out=pt[:, mo : mo + 128],
                                            in_=pt[:, mo : mo + 128],
                                            pattern=[[1, 128]],
                                            compare_op=mybir.AluOpType.is_ge,
                                            fill=0.0,
                                            base=0,
                                            channel_multiplier=-1,
                                        )
                                    pts[(h, j)] = (pt, qoff, W)
                                    if j < 4 * qb and j % 2 == 1:
                                        # pair adjacent full-width P^T tiles so
                                        # the rowsum matmul runs once per pair
                                        s2 = s2_pool.tile([128, 512], F32R)
                                        nc.vector.tensor_add(
                                            s2[:], pts[(h, j - 1)][0][:], pt[:]
                                        )
                                        s2s[(h, j // 2)] = s2
                            if jj >= PD:
                                j2 = jj - PD
                                for h in range(HPC):
                                    pt2, qoff2, W2 = pts.pop((h, j2))
                                    nc.tensor.matmul(
                                        o_ps[h][:, qoff2:512],
                                        v_sb[:, b * SC + j2, :],
                                        pt2[:, 0:W2],
                                        start=(j2 == 0),
                                        stop=(j2 == nj - 1),
                                        skip_group_check=True,
                                    )
                                    if j2 < 4 * qb:
                                        if j2 % 2 == 1:
                                            s2c = s2s.pop((h, j2 // 2))
                                            nc.tensor.matmul(
                                                r_ps[h][:],
                                                ones_sb[:],
                                                s2c[:],
                                                start=(j2 == first_rs),
                                                stop=False,
                                                skip_group_check=True,
                                            )
                                    else:
                                        nc.tensor.matmul(
                                            r_ps[h][:, qoff2:512],
                                            ones_sb[:],
                                            pt2[:, 0:W2],
                                            start=(j2 == first_rs),
                                            stop=(j2 == nj - 1),
                                            skip_group_check=True,
                                        )
                            emit_op(1)
                        rrs = {}
                        for h in range(HPC):
                            rrs[h] = rr_sb_pool.tile(
                                [128, 512], F32, tag="rr", name=f"rr{h}"
                            )
                            nc.vector.reciprocal(rrs[h][:], r_ps[h][:])
                        for stq in range(4):
                            sl = slice(stq * 128, (stq + 1) * 128)
                            for h in range(HPC):
                                nc.vector.tensor_mul(
                                    outT[
                                        :,
                                        h,
                                        qb * 512 + stq * 128 : qb * 512
                                        + (stq + 1) * 128,
                                    ],
                                    o_ps[h][:, sl],
                                    rrs[h][:, sl],
                                )
                            for nb in range(4):
                                pending.append((b, outT, qb * 4 + stq, nb))
                emit_op(len(pending))
    nc.compile()
    return nc


_GRAPH = None


def _rope_tables():
    inv_freq = 1.0 / (10000.0 ** (np.arange(0, D, 2, dtype=np.float32) / D))
    t = np.arange(S, dtype=np.float32)
    freqs = np.outer(t, inv_freq)
    emb = np.concatenate([freqs, freqs], axis=-1)  # (S, D)
    return (
        np.ascontiguousarray(np.cos(emb).T.astype(np.float32)),
        np.ascontiguousarray(np.sin(emb).T.astype(np.float32)),
    )


def kernel(x, wq, wk, wv, wo):
    global _GRAPH, LAST_EXEC_TIME_NS, LAST_RESULTS
    x = np.asarray(x, dtype=np.float32)
    wq = np.asarray(wq, dtype=np.float32)
    wk = np.asarray(wk, dtype=np.float32)
    wv = np.asarray(wv, dtype=np.float32)
    wo = np.asarray(wo, dtype=np.float32)

    xT = np.ascontiguousarray(x.reshape(R, H).T)
    cosT, sinT = _rope_tables()
    rotM = np.zeros((D, D), dtype=np.float32)
    for i in range(64):
        rotM[i, 64 + i] = -1.0
        rotM[64 + i, i] = 1.0
    rotMT = np.ascontiguousarray(rotM.T)
    ones_arr = np.ones((D, D), dtype=np.float32)
    scale = np.float32(1.0 / np.sqrt(D))

    in_maps = []
    for c in range(NCORES):
        kv = c // HPC
        wq_c = wq[:, c * HPC * D : (c + 1) * HPC * D] * scale
        wk_c = wk[:, kv * D : (kv + 1) * D]
        wv_c = wv[:, kv * D : (kv + 1) * D]
        wqkv_c = np.ascontiguousarray(
            np.concatenate([wq_c, wk_c, wv_c], axis=1, dtype=np.float32)
        )
        wo_c = np.ascontiguousarray(wo[c * HPC * D : (c + 1) * HPC * D, :])
        in_maps.append(
            {
                "xT": xT,
                "wqkv": wqkv_c,
                "wo": wo_c,
                "cosT": cosT,
                "sinT": sinT,
                "rotMT": rotMT,
                "onesd": ones_arr,
            }
        )

    if _GRAPH is None:
        _GRAPH = build_graph()

    # NTFF tracing is unavailable on axon clients without antenv.axon_hooks;
    # make sure an inherited BASS_TRACE can't break execution.
    os.environ["BASS_NEVER_TRACE"] = "1"
    res = None
    for attempt in range(3):
        try:
            res = run_bass_kernel_spmd(
                _GRAPH, in_maps, core_ids=list(range(NCORES))
            )
            break
        except Exception:
            # transient axon-terminal failures (mesh desync / LoadExecutable)
            # usually clear on retry
            if attempt == 2:
                raise
            time.sleep(5.0)
    LAST_EXEC_TIME_NS = res.exec_time_ns
    LAST_RESULTS = res
    acc = np.zeros((R, H), dtype=np.float32)
    for c in range(NCORES):
        acc += res.results[c]["out"]
    return acc.reshape(B, S, H)



# revision 74
# speedup vs baseline: 1.0912x; 1.0912x over previous
"""Trainium2 Bass kernel: GQA causal attention (B=2, S=2048, H=2048, 16 q-heads,
4 kv-heads, head_dim=128), tensor-parallel over 8 NeuronCores.

Sharding: 2 q-heads + their (shared) kv-head per core; wq/wk/wv column-sharded,
wo row-sharded.  Each core computes a partial o_proj output; the host sums the
8 partials (the standard TP partial-sum unshard).

All matmul operands are bf16 (PSUM accumulation stays fp32): same PE rate as
fp32r but half the DMA bytes, 2-4x DVE throughput on elementwise ops, and full
PE rate at any moving width (so causal tiles narrow to 128).

On-chip layouts are transposed (feature-on-partition) except V:
  q/k:   qkvT = w.T @ x.T            (PE, accumulate over 16 h-chunks)
  v:     natural [s, d] directly     (PE, xT chunks stationary, wv moving;
                                      no PE transposes needed)
  RoPE:  q' = q*cos + rot(q)*sin     (pure DVE: rot(q) via partition-offset
                                      muls against a sign-folded sin table)
  scoresT[k,q] = K @ Q^T             (PE; wq pre-scaled by 1/sqrt(D))
  P^T   = exp(scoresT - 40)          (ACT, fused bias; exact softmax after
                                      normalization: const cancels)
  causal mask: affine_select on P^T  (Pool/GpSimd, fill=0)
  outT  = V^T @ P^T                  (PE, PSUM-accumulated over k-chunks)
  rowsum: quad-packed ones-matmuls   (DVE pre-sums quads of P^T tiles so the
                                      PE streams 1/4 of the columns)
  outT *= bcast(1/rowsum)            (DVE recip/mul)
  out_partial = outT.T @ wo_c        (PE; outT is already the needed lhsT)

DMAs are batched (whole x row-block / whole output row) to amortize the
~625ns-per-DMA HWDGE cost; o_proj psum->sbuf copies rotate across DVE, Pool
and ACT so no single engine becomes co-critical with the PE.
"""

import os
import sys
import time

import numpy as np

sys.path.insert(0, "/opt/trn_rl_repo")

from contextlib import ExitStack

import concourse.bass as bass
from concourse import bacc
import concourse.mybir as mybir
import concourse.tile as tile
from concourse.bass_utils import run_bass_kernel_spmd

F32 = mybir.dt.float32
BF16 = mybir.dt.bfloat16
AF = mybir.ActivationFunctionType
ALU = mybir.AluOpType

B, S, H = 2, 2048, 2048
NH, KVH, D = 16, 4, 128
NCORES = 8
HPC = NH // NCORES  # q heads per core = 2
R = B * S  # 4096 flattened rows
QKV_W = HPC * D + 2 * D  # 512 = [q0|q1|k|v] columns per core
NB_RB = R // 512  # 8 row-blocks of 512
NB_HC = H // 128  # 16 contraction chunks
SB = S // 512  # 4 q-blocks per batch
SC = S // 128  # 16 k-chunks per batch
EXP_BIAS = -40.0

LAST_EXEC_TIME_NS = None
LAST_RESULTS = None


def build_graph(reps=1):
    nc = bacc.Bacc(
        "TRN2", target_bir_lowering=False, debug=False, num_devices=NCORES
    )
    # host-prepared layouts (see kernel()): xTr[rb*128+p, hc*512+c] =
    # x.T[hc*128+p, rb*512+c]; wqkvr[p, hc*512+c] = wqkv[hc*128+p, c];
    # wor[p, h*2048+c] = wo[h*128+p, c].
    xTr = nc.dram_tensor("xTr", [NB_RB * 128, NB_HC * 512], BF16, kind="ExternalInput").ap()
    wqkvr = nc.dram_tensor("wqkvr", [128, NB_HC * 512], BF16, kind="ExternalInput").ap()
    wor = nc.dram_tensor("wor", [128, HPC * H], BF16, kind="ExternalInput").ap()
    cosT = nc.dram_tensor("cosT", [D, S], BF16, kind="ExternalInput").ap()
    sinadjT = nc.dram_tensor("sinadjT", [D, S], BF16, kind="ExternalInput").ap()
    out = nc.dram_tensor("out", [R, H], BF16, kind="ExternalOutput").ap()

    with tile.TileContext(nc) as tc, ExitStack() as ctx:
        # ---- persistent SBUF ----
        const_pool = ctx.enter_context(tc.tile_pool(name="const", bufs=1))
        w_sb = const_pool.tile([128, NB_HC * 512], BF16)
        wo_sb = const_pool.tile([128, HPC * H], BF16)
        cos_sb = const_pool.tile([128, S], BF16)
        sinadj_sb = const_pool.tile([128, S], BF16)
        ones_sb = const_pool.tile([128, 128], BF16)  # rowsum lhsT / bcast
        expb_sb = const_pool.tile([128, 1], F32)  # exp bias (per-partition)
        # qk/v live in per-row-block tiles: tile-granular dependency tracking
        # would otherwise serialize attention's first reads behind the LAST
        # row-block's RoPE/copy on the DVE queue.
        qk_sb = {
            (cg, rb): const_pool.tile([128, 512], BF16, name=f"qk{cg}_{rb}")
            for cg in range(3)
            for rb in range(NB_RB)
        }
        v_sb = {
            rb: const_pool.tile([128, 512], BF16, name=f"v{rb}")
            for rb in range(NB_RB)
        }
        scr_sb = const_pool.tile([128, 1], F32)

        nc.gpsimd.memset(ones_sb[:], 1.0)
        nc.gpsimd.memset(expb_sb[:], EXP_BIAS)
        # touch Exp once so the ACT table load happens while ACT is idle,
        # not in front of the first real softmax tile
        nc.scalar.activation(scr_sb[:], expb_sb[:], AF.Exp, bias=0.0, scale=1.0)

        outT_pool = ctx.enter_context(tc.tile_pool(name="outT", bufs=2))
        ms_ps_pool = ctx.enter_context(tc.tile_pool(name="ms_ps", bufs=2, space="PSUM"))
        osb_pool = ctx.enter_context(tc.tile_pool(name="osb", bufs=6))
        xt_pool = ctx.enter_context(tc.tile_pool(name="xt", bufs=2))
        # scratch SBUF pools are persistent: per-phase pools would reuse the
        # same addresses and stall each phase's first ops on the previous
        # phase's last frees
        rtmp_pool = ctx.enter_context(tc.tile_pool(name="rtmp", bufs=8))
        pt_pool = ctx.enter_context(tc.tile_pool(name="pt", bufs=14))
        s2_pool = ctx.enter_context(tc.tile_pool(name="s2", bufs=6))
        s4_pool = ctx.enter_context(tc.tile_pool(name="s4", bufs=8))
        rr_sb_pool = ctx.enter_context(tc.tile_pool(name="rr_sb", bufs=2))

        # ---- o_proj drip FIFO: one (row-block, nb) pair per emission so the
        # in-order PE queue always has other matmuls between an o_proj pair
        # and its psum-slot dependency (the psum->sbuf copy). Output rows are
        # staged in a [128, 2048] row buffer and DMA'd once per row-block.
        pending = []
        ncopy = [0]

        def emit_op(nmax, split=False, pool=None, defer_below=0):
            for _ in range(nmax):
                if len(pending) <= defer_below:
                    return
                ob, oT, st, nb = pending.pop(0)
                op_ps = (pool or ms_ps_pool).tile(
                    [128, 512], F32, tag="ms", name="op_ps"
                )
                for h in range(HPC):
                    nc.tensor.matmul(
                        op_ps[:],
                        oT[:, h * S + st * 128 : h * S + (st + 1) * 128],
                        wo_sb[:, h * H + nb * 512 : h * H + (nb + 1) * 512],
                        start=(h == 0),
                        stop=(h == HPC - 1),
                    )
                osb = osb_pool.tile([128, 512], BF16, tag="osb", name="osb")
                if split:  # tail flush: alternate engines per tile
                    if ncopy[0] % 2 == 0:
                        nc.vector.tensor_copy(osb[:], op_ps[:])
                    else:
                        nc.scalar.copy(osb[:], op_ps[:])
                else:
                    # psum drains rotate 2:1 over DVE and ACT (ACT also
                    # carries the softmax exps; Pool cannot read PSUM)
                    if ncopy[0] % 3 == 1:
                        nc.scalar.copy(osb[:], op_ps[:])
                    else:
                        nc.vector.tensor_copy(osb[:], op_ps[:])
                ncopy[0] += 1
                r0 = ob * S + st * 128
                nc.sync.dma_start(
                    out[r0 : r0 + 128, nb * 512 : (nb + 1) * 512], osb[:]
                )

        xts = {}

        def fetch_xt(rb, granularity):
            t = xt_pool.tile([128, NB_HC * 512], BF16, tag="xt")
            step = NB_HC // granularity
            for g in range(granularity):
                sl = slice(g * step * 512, (g + 1) * step * 512)
                nc.sync.dma_start(t[:, sl], xTr[rb * 128 : (rb + 1) * 128, sl])
            xts[rb] = t

        for _rep in range(reps):
            for b in range(B):
                # ---- phase 1: qkvT matmuls + RoPE (q/k) + natural V ----
                with (
                    tc.tile_pool(name="qkv_ps", bufs=5, space="PSUM") as qkv_ps_pool,
                ):
                    for rbl in range(SB):
                        rb = b * SB + rbl
                        if rb == 0:
                            # startup: stream w and x at fine granularity so
                            # the first matmuls' deps land early; rope tables
                            # right after the first chunks.
                            t = xt_pool.tile([128, NB_HC * 512], BF16, tag="xt")
                            xts[0] = t
                            for lo, hi in [(0, 1), (1, 2), (2, 4), (4, 6),
                                           (6, 8), (8, 10), (10, 12), (12, 14),
                                           (14, 16)]:
                                sl = slice(lo * 512, hi * 512)
                                nc.sync.dma_start(w_sb[:, sl], wqkvr[:, sl])
                                nc.sync.dma_start(t[:, sl], xTr[0:128, sl])
                        xt = xts.pop(rb)
                        if rbl + 1 < SB:
                            fetch_xt(rb + 1, 2)
                        elif b + 1 < B:
                            fetch_xt((b + 1) * SB, 2)
                        if rb == 0:
                            # rope tables + wo after the next row-block's x:
                            # they are only needed once DMA has caught up
                            nc.sync.dma_start(cos_sb[:], cosT)
                            nc.sync.dma_start(sinadj_sb[:], sinadjT)
                            nc.sync.dma_start(wo_sb[:], wor)
                        q0_ps = qkv_ps_pool.tile([128, 512], F32, tag="qkvps", name="q0")
                        q1_ps = qkv_ps_pool.tile([128, 512], F32, tag="qkvps", name="q1")
                        k_ps = qkv_ps_pool.tile([128, 512], F32, tag="qkvps", name="k")
                        v_ps = qkv_ps_pool.tile([128, 512], F32, tag="qkvps", name="v")
                        qk_list = [q0_ps, q1_ps, k_ps]
                        for hc in range(NB_HC):
                            xsl = xt[:, hc * 512 : (hc + 1) * 512]
                            for cg in range(3):
                                nc.tensor.matmul(
                                    qk_list[cg][:],
                                    w_sb[:, hc * 512 + cg * 128 : hc * 512 + (cg + 1) * 128],
                                    xsl,
                                    start=(hc == 0),
                                    stop=(hc == NB_HC - 1),
                                )
                            emit_op(1)
                        # v: one accumulation group at a time — interleaving
                        # independent start/stop groups in different column
                        # regions of one PSUM bank miscomputes on HW
                        for rc in range(4):
                            for hc in range(NB_HC):
                                nc.tensor.matmul(
                                    v_ps[:, rc * 128 : (rc + 1) * 128],
                                    xt[:, hc * 512 + rc * 128 : hc * 512 + (rc + 1) * 128],
                                    w_sb[:, hc * 512 + 384 : hc * 512 + 512],
                                    start=(hc == 0),
                                    stop=(hc == NB_HC - 1),
                                )
                            emit_op(1)
                        # RoPE: q' = q*cos + rot(q)*sin, with rot via
                        # partition-offset muls against the sign-folded table
                        # (sinadj[0:64] = -sin[0:64], sinadj[64:128] = +sin).
                        # q0/q1 on DVE; the k head + the V psum drain go to
                        # the otherwise-idle Pool engine.
                        cs = cos_sb[:, rbl * 512 : (rbl + 1) * 512]
                        sn_lo = sinadj_sb[0:64, rbl * 512 : (rbl + 1) * 512]
                        sn_hi = sinadj_sb[64:128, rbl * 512 : (rbl + 1) * 512]
                        # Drain each psum bank with a single bf16 copy on the
                        # phase-1-idle ACT engine so banks free fast, then do
                        # the rope math from the SBUF copies at bf16
                        # throughput: q0/q1 on DVE, k on Pool (SBUF-only ops).
                        nc.scalar.copy(v_sb[rb][:], v_ps[:])
                        raws = {}
                        for cg in (2, 0, 1):
                            qraw = rtmp_pool.tile([128, 512], BF16, tag="rtmp")
                            nc.scalar.copy(qraw[:], qk_list[cg][:])
                            raws[cg] = qraw
                        for cg in (2, 0, 1):
                            # rotate-half reads come partition-offset straight
                            # from PSUM (the equal-base rule only binds when
                            # both inputs are SBUF); psum reads go first so
                            # the bank frees as early as possible, k head
                            # first since its bank unblocks attention scores
                            ps = qk_list[cg]
                            qraw = raws[cg]
                            t1 = rtmp_pool.tile([128, 512], BF16, tag="rtmp")
                            t2 = rtmp_pool.tile([128, 512], BF16, tag="rtmp")
                            nc.vector.tensor_mul(t2[0:64, :], ps[64:128, :], sn_lo)
                            nc.vector.tensor_mul(t2[64:128, :], ps[0:64, :], sn_hi)
                            nc.vector.tensor_mul(t1[:], qraw[:], cs)
                            nc.vector.tensor_add(qk_sb[(cg, rb)][:], t1[:], t2[:])

                # ---- phase 2: attention for batch b ----
                # pool-open order controls bank placement: rs (written last)
                # takes the banks freed last by phase 1; st (needed first)
                # lands on the earliest-freed/spare banks
                with (
                    tc.tile_pool(name="rs_ps", bufs=2, space="PSUM") as rs_ps_pool,
                    tc.tile_pool(name="ot_ps", bufs=2, space="PSUM") as ot_ps_pool,
                    tc.tile_pool(name="st_ps", bufs=2, space="PSUM") as st_ps_pool,
                ):
                    PD = 5  # per-head pipeline depth
                    outT = outT_pool.tile([128, HPC * S], BF16)
                    for qb in range(SB):
                        o_ps, r_ps = {}, {}
                        for h in range(HPC):
                            o_ps[h] = ot_ps_pool.tile(
                                [128, 512], F32, tag="ot", name=f"ot{h}"
                            )
                            r_ps[h] = rs_ps_pool.tile(
                                [128, 512], F32, tag="rs", name=f"rs{h}"
                            )
                        nj = 4 * qb + 4
                        pd = 3 if (b == B - 1 and qb == SB - 1) else PD
                        # before the last batch, defer most o_proj drips into
                        # the next phase 1, whose DVE/ACT are mostly idle —
                        # in attention both engines are near-saturated
                        dfb = 20 if b < B - 1 else 0
                        pts = {}
                        s2s = {}
                        s4s = {}
                        diag = {}
                        for jj in range(nj + pd):
                            emit_op(2 if jj < 2 else 1, defer_below=dfb)
                            if jj < nj:
                                j = jj
                                r = j - 4 * qb  # diagonal band index
                                qoff = 128 * r if r > 0 else 0
                                W = 512 - qoff
                                for h in range(HPC):
                                    s_ps = st_ps_pool.tile([128, 512], F32)
                                    nc.tensor.matmul(
                                        s_ps[:, qoff:512],
                                        qk_sb[(2, b * SB + j // 4)][
                                            :, (j % 4) * 128 : (j % 4 + 1) * 128
                                        ],
                                        qk_sb[(h, b * SB + qb)][:, qoff:512],
                                        start=True,
                                        stop=True,
                                    )
                                    pt = pt_pool.tile([128, 512], BF16)
                                    nc.scalar.activation(
                                        pt[:, qoff:512],
                                        s_ps[:, qoff:512],
                                        AF.Exp,
                                        bias=expb_sb[:],
                                        scale=1.0,
                                    )
                                    if r >= 0:
                                        # zero where k > q inside the 128-wide
                                        # diagonal ramp
                                        nc.gpsimd.affine_select(
                                            out=pt[:, qoff : qoff + 128],
                                            in_=pt[:, qoff : qoff + 128],
                                            pattern=[[1, 128]],
                                            compare_op=ALU.is_ge,
                                            fill=0.0,
                                            base=0,
                                            channel_multiplier=-1,
                                        )
                                    pts[(h, j)] = (pt, qoff, W)
                                    # rowsum packing on DVE (all-bf16 = fast):
                                    padd = nc.vector.tensor_add
                                    pcopy = nc.vector.tensor_copy
                                    if j < 4 * qb:
                                        if j % 2 == 1:
                                            s2 = s2_pool.tile([128, 512], BF16, tag="s2")
                                            padd(s2[:], pts[(h, j - 1)][0][:], pt[:])
                                            s2s[(h, j // 2)] = s2
                                        if j % 4 == 3:
                                            s4 = s4_pool.tile([128, 512], BF16, tag="s4")
                                            padd(
                                                s4[:],
                                                s2s.pop((h, j // 2 - 1))[:],
                                                s2s.pop((h, j // 2))[:],
                                            )
                                            s4s[(h, j // 4)] = s4
                                    elif r == 1:
                                        pt0 = pts[(h, 4 * qb)][0]
                                        sa = s4_pool.tile([128, 512], BF16, tag="s4")
                                        pcopy(sa[:, 0:128], pt0[:, 0:128])
                                        padd(
                                            sa[:, 128:512],
                                            pt0[:, 128:512],
                                            pt[:, 128:512],
                                        )
                                        diag[(h, 0)] = sa
                                    elif r == 3:
                                        pt2 = pts[(h, 4 * qb + 2)][0]
                                        sb_ = s4_pool.tile([128, 512], BF16, tag="s4")
                                        pcopy(sb_[:, 256:384], pt2[:, 256:384])
                                        padd(
                                            sb_[:, 384:512],
                                            pt2[:, 384:512],
                                            pt[:, 384:512],
                                        )
                                        diag[(h, 1)] = sb_
                            if jj >= pd:
                                j2 = jj - pd
                                for h in range(HPC):
                                    pt2, qoff2, W2 = pts.pop((h, j2))
                                    if j2 < 4 * qb:
                                        if j2 % 4 == 3:
                                            s4c = s4s.pop((h, j2 // 4))
                                            nc.tensor.matmul(
                                                r_ps[h][:],
                                                ones_sb[:],
                                                s4c[:],
                                                start=(j2 == 3),
                                                stop=False,
                                                skip_group_check=True,
                                            )
                                    elif j2 == 4 * qb + 1:
                                        nc.tensor.matmul(
                                            r_ps[h][:],
                                            ones_sb[:],
                                            diag[(h, 0)][:],
                                            start=(qb == 0),
                                            stop=False,
                                            skip_group_check=True,
                                        )
                                    elif j2 == 4 * qb + 3:
                                        nc.tensor.matmul(
                                            r_ps[h][:, 256:512],
                                            ones_sb[:],
                                            diag[(h, 1)][:, 256:512],
                                            start=False,
                                            stop=True,
                                            skip_group_check=True,
                                        )
                                    nc.tensor.matmul(
                                        o_ps[h][:, qoff2:512],
                                        v_sb[b * SB + j2 // 4][
                                            :, (j2 % 4) * 128 : (j2 % 4 + 1) * 128
                                        ],
                                        pt2[:, qoff2:512],
                                        start=(j2 == 0),
                                        stop=(j2 == nj - 1),
                                        skip_group_check=True,
                                    )
                            emit_op(1, defer_below=dfb)
                        for h in range(HPC):
                            rr = rr_sb_pool.tile([128, 512], F32, tag="rr")
                            nc.vector.reciprocal(rr[:], r_ps[h][:])
                            nc.vector.tensor_mul(
                                outT[:, h * S + qb * 512 : h * S + (qb + 1) * 512],
                                o_ps[h][:],
                                rr[:],
                            )
                        for stq in range(4):
                            for nb in range(4):
                                pending.append((b, outT, qb * 4 + stq, nb))
            # final drain: attention pools are closed, so spread the o_proj
            # psums over 6 banks to decouple the matmul stream from the
            # copy+DMA latency chain
            with tc.tile_pool(name="drain_ps", bufs=6, space="PSUM") as drain_pool:
                emit_op(len(pending), split=True, pool=drain_pool)
    nc.compile()
    return nc


_GRAPH = None


def _rope_tables():
    inv_freq = 1.0 / (10000.0 ** (np.arange(0, D, 2, dtype=np.float32) / D))
    t = np.arange(S, dtype=np.float32)
    freqs = np.outer(t, inv_freq)
    emb = np.concatenate([freqs, freqs], axis=-1)  # (S, D)
    cosT = np.ascontiguousarray(np.cos(emb).T.astype(np.float32))
    sinT = np.ascontiguousarray(np.sin(emb).T.astype(np.float32))
    sinadjT = sinT.copy()
    sinadjT[0:64, :] *= -1.0  # fold rotate_half's sign into the table
    return cosT, sinadjT


def kernel(x, wq, wk, wv, wo):
    global _GRAPH, LAST_EXEC_TIME_NS, LAST_RESULTS
    import ml_dtypes

    bf16 = ml_dtypes.bfloat16
    x = np.asarray(x, dtype=np.float32)
    wq = np.asarray(wq, dtype=np.float32)
    wk = np.asarray(wk, dtype=np.float32)
    wv = np.asarray(wv, dtype=np.float32)
    wo = np.asarray(wo, dtype=np.float32)

    xT = np.ascontiguousarray(x.reshape(R, H).T)
    # xTr[rb*128+p, hc*512+c] = xT[hc*128+p, rb*512+c]
    xTr = np.ascontiguousarray(
        xT.reshape(NB_HC, 128, NB_RB, 512).transpose(2, 1, 0, 3).reshape(
            NB_RB * 128, NB_HC * 512
        )
    ).astype(bf16)
    cosT, sinadjT = _rope_tables()
    cosT = cosT.astype(bf16)
    sinadjT = sinadjT.astype(bf16)
    scale = np.float32(1.0 / np.sqrt(D))

    in_maps = []
    for c in range(NCORES):
        kv = c // HPC
        wq_c = wq[:, c * HPC * D : (c + 1) * HPC * D] * scale
        wk_c = wk[:, kv * D : (kv + 1) * D]
        wv_c = wv[:, kv * D : (kv + 1) * D]
        wqkv_c = np.concatenate([wq_c, wk_c, wv_c], axis=1, dtype=np.float32)
        # wqkvr[p, hc*512+c] = wqkv_c[hc*128+p, c]
        wqkvr = np.ascontiguousarray(
            wqkv_c.reshape(NB_HC, 128, QKV_W).transpose(1, 0, 2).reshape(
                128, NB_HC * 512
            )
        ).astype(bf16)
        wo_c = wo[c * HPC * D : (c + 1) * HPC * D, :]
        wor = np.ascontiguousarray(
            wo_c.reshape(HPC, 128, H).transpose(1, 0, 2).reshape(128, HPC * H)
        ).astype(bf16)
        in_maps.append(
            {
                "xTr": xTr,
                "wqkvr": wqkvr,
                "wor": wor,
                "cosT": cosT,
                "sinadjT": sinadjT,
            }
        )

    if _GRAPH is None:
        _GRAPH = build_graph()

    # NTFF tracing is unavailable on axon clients without antenv.axon_hooks;
    # make sure an inherited BASS_TRACE can't break execution.
    os.environ["BASS_NEVER_TRACE"] = "1"
    res = None
    for attempt in range(3):
        try:
            res = run_bass_kernel_spmd(
                _GRAPH, in_maps, core_ids=list(range(NCORES))
            )
            break
        except Exception:
            # transient axon-terminal failures (mesh desync / LoadExecutable)
            # usually clear on retry
            if attempt == 2:
                raise
            time.sleep(5.0)
    LAST_EXEC_TIME_NS = res.exec_time_ns
    LAST_RESULTS = res
    acc = np.zeros((R, H), dtype=np.float32)
    for c in range(NCORES):
        acc += np.asarray(res.results[c]["out"], dtype=np.float32)
    return acc.reshape(B, S, H)


# revision 80
# speedup vs baseline: 1.1243x; 1.0303x over previous
"""Trainium2 Bass kernel: GQA causal attention (B=2, S=2048, H=2048, 16 q-heads,
4 kv-heads, head_dim=128), tensor-parallel over 8 NeuronCores.

Sharding: 2 q-heads + their (shared) kv-head per core; wq/wk/wv column-sharded,
wo row-sharded.  Each core computes a partial o_proj output; the host sums the
8 partials (the standard TP partial-sum unshard).

All matmul operands are bf16 (PSUM accumulation stays fp32): same PE rate as
fp32r but half the DMA bytes, 2-4x DVE throughput on elementwise ops, and full
PE rate at any moving width (so causal tiles narrow to 128).

On-chip layouts are transposed (feature-on-partition) except V:
  q/k:   qkvT = w.T @ x.T            (PE, accumulate over 16 h-chunks)
  v:     natural [s, d] directly     (PE, xT chunks stationary, wv moving;
                                      no PE transposes needed)
  RoPE:  q' = q*cos + rot(q)*sin     (pure DVE: rot(q) via partition-offset
                                      muls against a sign-folded sin table)
  scoresT[k,q] = K @ Q^T             (PE; wq pre-scaled by 1/sqrt(D))
  P^T   = exp(scoresT - 40)          (ACT, fused bias; exact softmax after
                                      normalization: const cancels)
  causal mask: affine_select on P^T  (Pool/GpSimd, fill=0)
  outT  = V^T @ P^T                  (PE, PSUM-accumulated over k-chunks)
  rowsum: quad-packed ones-matmuls   (DVE pre-sums quads of P^T tiles so the
                                      PE streams 1/4 of the columns)
  outT *= bcast(1/rowsum)            (DVE recip/mul)
  out_partial = outT.T @ wo_c        (PE; outT is already the needed lhsT)

DMAs are batched (whole x row-block) to amortize the ~625ns-per-DMA HWDGE
cost; o_proj psum->sbuf drains rotate 2:1 over DVE and ACT (Pool cannot read
PSUM), and before the last batch most o_proj drips are deferred into the next
projection phase, whose DVE/ACT are otherwise idle.

HW-verified constraints honored here (the cost model does not flag them):
interleaved independent PSUM accumulation groups in one bank miscompute (so
V's four column-region groups run sequentially), and same-partition-base is
required when both DVE inputs live in SBUF (so rotate-half reads PSUM).
"""

import os
import sys
import time

import numpy as np

sys.path.insert(0, "/opt/trn_rl_repo")

from contextlib import ExitStack

import concourse.bass as bass
from concourse import bacc
import concourse.mybir as mybir
import concourse.tile as tile
from concourse.bass_utils import run_bass_kernel_spmd

F32 = mybir.dt.float32
BF16 = mybir.dt.bfloat16
AF = mybir.ActivationFunctionType
ALU = mybir.AluOpType

B, S, H = 2, 2048, 2048
NH, KVH, D = 16, 4, 128
NCORES = 8
HPC = NH // NCORES  # q heads per core = 2
R = B * S  # 4096 flattened rows
QKV_W = HPC * D + 2 * D  # 512 = [q0|q1|k|v] columns per core
NB_RB = R // 512  # 8 row-blocks of 512
NB_HC = H // 128  # 16 contraction chunks
SB = S // 512  # 4 q-blocks per batch
SC = S // 128  # 16 k-chunks per batch
EXP_BIAS = -40.0

LAST_EXEC_TIME_NS = None
LAST_RESULTS = None


def build_graph(reps=1):
    nc = bacc.Bacc(
        "TRN2", target_bir_lowering=False, debug=False, num_devices=NCORES
    )
    # host-prepared layouts (see kernel()): xTr[rb*128+p, hc*512+c] =
    # x.T[hc*128+p, rb*512+c]; wqkvr[p, hc*512+c] = wqkv[hc*128+p, c];
    # wor[p, h*2048+c] = wo[h*128+p, c].
    xTr = nc.dram_tensor("xTr", [NB_RB * 128, NB_HC * 512], BF16, kind="ExternalInput").ap()
    wqkvr = nc.dram_tensor("wqkvr", [128, NB_HC * 512], BF16, kind="ExternalInput").ap()
    wor = nc.dram_tensor("wor", [128, HPC * H], BF16, kind="ExternalInput").ap()
    cosT = nc.dram_tensor("cosT", [D, S], BF16, kind="ExternalInput").ap()
    sinadjT = nc.dram_tensor("sinadjT", [D, S], BF16, kind="ExternalInput").ap()
    out = nc.dram_tensor("out", [R, H], BF16, kind="ExternalOutput").ap()

    with tile.TileContext(nc) as tc, ExitStack() as ctx:
        # ---- persistent SBUF ----
        const_pool = ctx.enter_context(tc.tile_pool(name="const", bufs=1))
        w_sb = const_pool.tile([128, NB_HC * 512], BF16)
        wo_sb = const_pool.tile([128, HPC * H], BF16)
        cos_sb = const_pool.tile([128, S], BF16)
        sinadj_sb = const_pool.tile([128, S], BF16)
        ones_sb = const_pool.tile([128, 128], BF16)  # rowsum lhsT / bcast
        expb_sb = const_pool.tile([128, 1], F32)  # exp bias (per-partition)
        # qk/v live in per-row-block tiles: tile-granular dependency tracking
        # would otherwise serialize attention's first reads behind the LAST
        # row-block's RoPE/copy on the DVE queue.
        qk_sb = {
            (cg, rb): const_pool.tile([128, 512], BF16, name=f"qk{cg}_{rb}")
            for cg in range(3)
            for rb in range(NB_RB)
        }
        v_sb = {
            rb: const_pool.tile([128, 512], BF16, name=f"v{rb}")
            for rb in range(NB_RB)
        }
        scr_sb = const_pool.tile([128, 1], F32)

        nc.gpsimd.memset(ones_sb[:], 1.0)
        nc.gpsimd.memset(expb_sb[:], EXP_BIAS)
        # touch Exp once so the ACT table load happens while ACT is idle,
        # not in front of the first real softmax tile
        nc.scalar.activation(scr_sb[:], expb_sb[:], AF.Exp, bias=0.0, scale=1.0)

        outT_pool = ctx.enter_context(tc.tile_pool(name="outT", bufs=2))
        ms_ps_pool = ctx.enter_context(tc.tile_pool(name="ms_ps", bufs=2, space="PSUM"))
        osb_pool = ctx.enter_context(tc.tile_pool(name="osb", bufs=6))
        xt_pool = ctx.enter_context(tc.tile_pool(name="xt", bufs=2))
        # scratch SBUF pools are persistent: per-phase pools would reuse the
        # same addresses and stall each phase's first ops on the previous
        # phase's last frees
        rtmp_pool = ctx.enter_context(tc.tile_pool(name="rtmp", bufs=8))
        pt_pool = ctx.enter_context(tc.tile_pool(name="pt", bufs=14))
        s2_pool = ctx.enter_context(tc.tile_pool(name="s2", bufs=6))
        s4_pool = ctx.enter_context(tc.tile_pool(name="s4", bufs=8))
        rr_sb_pool = ctx.enter_context(tc.tile_pool(name="rr_sb", bufs=2))

        # ---- o_proj drip FIFO: one (row-block, nb) pair per emission so the
        # in-order PE queue always has other matmuls between an o_proj pair
        # and its psum-slot dependency (the psum->sbuf copy). Output rows are
        # staged in a [128, 2048] row buffer and DMA'd once per row-block.
        pending = []
        ncopy = [0]
        osb_rows = {}

        def emit_op(nmax, split=False, pool=None, defer_below=0):
            for _ in range(nmax):
                if len(pending) <= defer_below:
                    return
                ob, oT, st, nb = pending.pop(0)
                op_ps = (pool or ms_ps_pool).tile(
                    [128, 512], F32, tag="ms", name="op_ps"
                )
                for h in range(HPC):
                    nc.tensor.matmul(
                        op_ps[:],
                        oT[:, h * S + st * 128 : h * S + (st + 1) * 128],
                        wo_sb[:, h * H + nb * 512 : h * H + (nb + 1) * 512],
                        start=(h == 0),
                        stop=(h == HPC - 1),
                    )
                r0 = ob * S + st * 128
                if split:
                    # tail flush: copies alternate engines per tile and land
                    # in a row buffer, so the trailing out-writes are 4 wide
                    # DMAs instead of 16 HWDGE dispatches
                    key = (ob, st)
                    if nb == 0:
                        osb_rows[key] = osb_pool.tile(
                            [128, H], BF16, tag="orow", name="orow"
                        )
                    orow = osb_rows[key]
                    dst = orow[:, nb * 512 : (nb + 1) * 512]
                    if ncopy[0] % 2 == 0:
                        nc.vector.tensor_copy(dst, op_ps[:])
                    else:
                        nc.scalar.copy(dst, op_ps[:])
                    if nb == 3:
                        del osb_rows[key]
                        nc.sync.dma_start(out[r0 : r0 + 128, :], orow[:])
                else:
                    osb = osb_pool.tile([128, 512], BF16, tag="osb", name="osb")
                    # psum drains rotate 2:1 over DVE and ACT (ACT also
                    # carries the softmax exps; Pool cannot read PSUM)
                    if ncopy[0] % 3 == 1:
                        nc.scalar.copy(osb[:], op_ps[:])
                    else:
                        nc.vector.tensor_copy(osb[:], op_ps[:])
                    nc.sync.dma_start(
                        out[r0 : r0 + 128, nb * 512 : (nb + 1) * 512], osb[:]
                    )
                ncopy[0] += 1

        xts = {}

        def fetch_xt(rb, granularity):
            t = xt_pool.tile([128, NB_HC * 512], BF16, tag="xt")
            step = NB_HC // granularity
            for g in range(granularity):
                sl = slice(g * step * 512, (g + 1) * step * 512)
                nc.sync.dma_start(t[:, sl], xTr[rb * 128 : (rb + 1) * 128, sl])
            xts[rb] = t

        for _rep in range(reps):
            for b in range(B):
                # ---- phase 1: qkvT matmuls + RoPE (q/k) + natural V ----
                with (
                    tc.tile_pool(name="qkv_ps", bufs=5, space="PSUM") as qkv_ps_pool,
                ):
                    for rbl in range(SB):
                        rb = b * SB + rbl
                        if rb == 0:
                            # startup: stream w and x at fine granularity so
                            # the first matmuls' deps land early; rope tables
                            # right after the first chunks.
                            t = xt_pool.tile([128, NB_HC * 512], BF16, tag="xt")
                            xts[0] = t
                            for lo, hi in [(0, 1), (1, 2), (2, 4), (4, 6),
                                           (6, 8), (8, 10), (10, 12), (12, 14),
                                           (14, 16)]:
                                sl = slice(lo * 512, hi * 512)
                                nc.sync.dma_start(w_sb[:, sl], wqkvr[:, sl])
                                nc.sync.dma_start(t[:, sl], xTr[0:128, sl])
                        xt = xts.pop(rb)
                        if rbl + 1 < SB:
                            fetch_xt(rb + 1, 2)
                        elif b + 1 < B:
                            fetch_xt((b + 1) * SB, 2)
                        if rb == 0:
                            # rope tables + wo after the next row-block's x:
                            # they are only needed once DMA has caught up
                            nc.sync.dma_start(cos_sb[:], cosT)
                            nc.sync.dma_start(sinadj_sb[:], sinadjT)
                            nc.sync.dma_start(wo_sb[:], wor)
                        # Each projection runs as ONE sequential accumulation
                        # group (k first, then q0/q1, then V's four column
                        # regions): each group's rope/drain starts while the
                        # next group is still on the PE, so by row-block end
                        # no psum-read backlog remains to stall the phase
                        # handoff. (Interleaving independent start/stop
                        # groups in one PSUM bank also miscomputes on HW.)
                        cs = cos_sb[:, rbl * 512 : (rbl + 1) * 512]
                        sn_lo = sinadj_sb[0:64, rbl * 512 : (rbl + 1) * 512]
                        sn_hi = sinadj_sb[64:128, rbl * 512 : (rbl + 1) * 512]

                        def rope_drain(ps, dst):
                            # drain the bank with one bf16 ACT copy, then
                            # rope on DVE: the rotate-half reads come
                            # partition-offset straight from PSUM (the
                            # equal-base rule only binds when both inputs are
                            # SBUF); psum reads go first so the bank frees
                            # as early as possible
                            qraw = rtmp_pool.tile(
                                [128, 512], BF16, tag="rtmp", name="qraw"
                            )
                            nc.scalar.copy(qraw[:], ps[:])
                            t1 = rtmp_pool.tile([128, 512], BF16, tag="rtmp", name="t1")
                            t2 = rtmp_pool.tile([128, 512], BF16, tag="rtmp", name="t2")
                            nc.vector.tensor_mul(t2[0:64, :], ps[64:128, :], sn_lo)
                            nc.vector.tensor_mul(t2[64:128, :], ps[0:64, :], sn_hi)
                            nc.vector.tensor_mul(t1[:], qraw[:], cs)
                            nc.vector.tensor_add(dst, t1[:], t2[:])

                        for cg, dst_key in ((2, 2), (0, 0), (1, 1)):
                            g_ps = qkv_ps_pool.tile(
                                [128, 512], F32, tag="qkvps", name=f"g{cg}"
                            )
                            for hc in range(NB_HC):
                                nc.tensor.matmul(
                                    g_ps[:],
                                    w_sb[:, hc * 512 + cg * 128 : hc * 512 + (cg + 1) * 128],
                                    xt[:, hc * 512 : (hc + 1) * 512],
                                    start=(hc == 0),
                                    stop=(hc == NB_HC - 1),
                                )
                                if hc % 4 == 1:
                                    emit_op(1)
                            rope_drain(g_ps, qk_sb[(dst_key, rb)][:])
                        v_ps = qkv_ps_pool.tile([128, 512], F32, tag="qkvps", name="v")
                        for rc in range(4):
                            for hc in range(NB_HC):
                                nc.tensor.matmul(
                                    v_ps[:, rc * 128 : (rc + 1) * 128],
                                    xt[:, hc * 512 + rc * 128 : hc * 512 + (rc + 1) * 128],
                                    w_sb[:, hc * 512 + 384 : hc * 512 + 512],
                                    start=(hc == 0),
                                    stop=(hc == NB_HC - 1),
                                )
                            emit_op(1)
                        nc.scalar.copy(v_sb[rb][:], v_ps[:])

                # ---- phase 2: attention for batch b ----
                # pool-open order controls bank placement: rs (written last)
                # takes the banks freed last by phase 1; st (needed first)
                # lands on the earliest-freed/spare banks
                with (
                    tc.tile_pool(name="rs_ps", bufs=2, space="PSUM") as rs_ps_pool,
                    tc.tile_pool(name="ot_ps", bufs=2, space="PSUM") as ot_ps_pool,
                    tc.tile_pool(name="st_ps", bufs=2, space="PSUM") as st_ps_pool,
                ):
                    PD = 5  # per-head pipeline depth
                    outT = outT_pool.tile([128, HPC * S], BF16)
                    for qb in range(SB):
                        o_ps, r_ps = {}, {}
                        for h in range(HPC):
                            o_ps[h] = ot_ps_pool.tile(
                                [128, 512], F32, tag="ot", name=f"ot{h}"
                            )
                            r_ps[h] = rs_ps_pool.tile(
                                [128, 512], F32, tag="rs", name=f"rs{h}"
                            )
                        nj = 4 * qb + 4
                        pd = 3 if (b == B - 1 and qb == SB - 1) else PD
                        # before the last batch, defer most o_proj drips into
                        # the next phase 1, whose DVE/ACT are mostly idle —
                        # in attention both engines are near-saturated
                        dfb = 20 if b < B - 1 else 0
                        pts = {}
                        s2s = {}
                        s4s = {}
                        diag = {}
                        for jj in range(nj + pd):
                            emit_op(2 if jj < 2 else 1, defer_below=dfb)
                            if jj < nj:
                                j = jj
                                r = j - 4 * qb  # diagonal band index
                                qoff = 128 * r if r > 0 else 0
                                W = 512 - qoff
                                for h in range(HPC):
                                    s_ps = st_ps_pool.tile([128, 512], F32)
                                    nc.tensor.matmul(
                                        s_ps[:, qoff:512],
                                        qk_sb[(2, b * SB + j // 4)][
                                            :, (j % 4) * 128 : (j % 4 + 1) * 128
                                        ],
                                        qk_sb[(h, b * SB + qb)][:, qoff:512],
                                        start=True,
                                        stop=True,
                                    )
                                    pt = pt_pool.tile([128, 512], BF16)
                                    nc.scalar.activation(
                                        pt[:, qoff:512],
                                        s_ps[:, qoff:512],
                                        AF.Exp,
                                        bias=expb_sb[:],
                                        scale=1.0,
                                    )
                                    if r >= 0:
                                        # zero where k > q inside the 128-wide
                                        # diagonal ramp
                                        nc.gpsimd.affine_select(
                                            out=pt[:, qoff : qoff + 128],
                                            in_=pt[:, qoff : qoff + 128],
                                            pattern=[[1, 128]],
                                            compare_op=ALU.is_ge,
                                            fill=0.0,
                                            base=0,
                                            channel_multiplier=-1,
                                        )
                                    pts[(h, j)] = (pt, qoff, W)
                                    # rowsum packing on DVE (all-bf16 = fast):
                                    padd = nc.vector.tensor_add
                                    pcopy = nc.vector.tensor_copy
                                    if j < 4 * qb:
                                        if j % 2 == 1:
                                            s2 = s2_pool.tile([128, 512], BF16, tag="s2")
                                            padd(s2[:], pts[(h, j - 1)][0][:], pt[:])
                                            s2s[(h, j // 2)] = s2
                                        if j % 4 == 3:
                                            s4 = s4_pool.tile([128, 512], BF16, tag="s4")
                                            padd(
                                                s4[:],
                                                s2s.pop((h, j // 2 - 1))[:],
                                                s2s.pop((h, j // 2))[:],
                                            )
                                            s4s[(h, j // 4)] = s4
                                    elif r == 1:
                                        pt0 = pts[(h, 4 * qb)][0]
                                        sa = s4_pool.tile([128, 512], BF16, tag="s4")
                                        pcopy(sa[:, 0:128], pt0[:, 0:128])
                                        padd(
                                            sa[:, 128:512],
                                            pt0[:, 128:512],
                                            pt[:, 128:512],
                                        )
                                        diag[(h, 0)] = sa
                                    elif r == 3:
                                        pt2 = pts[(h, 4 * qb + 2)][0]
                                        sb_ = s4_pool.tile([128, 512], BF16, tag="s4")
                                        pcopy(sb_[:, 256:384], pt2[:, 256:384])
                                        padd(
                                            sb_[:, 384:512],
                                            pt2[:, 384:512],
                                            pt[:, 384:512],
                                        )
                                        diag[(h, 1)] = sb_
                            if jj >= pd:
                                j2 = jj - pd
                                for h in range(HPC):
                                    pt2, qoff2, W2 = pts.pop((h, j2))
                                    if j2 < 4 * qb:
                                        if j2 % 4 == 3:
                                            s4c = s4s.pop((h, j2 // 4))
                                            nc.tensor.matmul(
                                                r_ps[h][:],
                                                ones_sb[:],
                                                s4c[:],
                                                start=(j2 == 3),
                                                stop=False,
                                                skip_group_check=True,
                                            )
                                    elif j2 == 4 * qb + 1:
                                        nc.tensor.matmul(
                                            r_ps[h][:],
                                            ones_sb[:],
                                            diag[(h, 0)][:],
                                            start=(qb == 0),
                                            stop=False,
                                            skip_group_check=True,
                                        )
                                    elif j2 == 4 * qb + 3:
                                        nc.tensor.matmul(
                                            r_ps[h][:, 256:512],
                                            ones_sb[:],
                                            diag[(h, 1)][:, 256:512],
                                            start=False,
                                            stop=True,
                                            skip_group_check=True,
                                        )
                                    nc.tensor.matmul(
                                        o_ps[h][:, qoff2:512],
                                        v_sb[b * SB + j2 // 4][
                                            :, (j2 % 4) * 128 : (j2 % 4 + 1) * 128
                                        ],
                                        pt2[:, qoff2:512],
                                        start=(j2 == 0),
                                        stop=(j2 == nj - 1),
                                        skip_group_check=True,
                                    )
                            emit_op(1, defer_below=dfb)
                        for h in range(HPC):
                            rr = rr_sb_pool.tile([128, 512], F32, tag="rr")
                            nc.vector.reciprocal(rr[:], r_ps[h][:])
                            nc.vector.tensor_mul(
                                outT[:, h * S + qb * 512 : h * S + (qb + 1) * 512],
                                o_ps[h][:],
                                rr[:],
                            )
                        for stq in range(4):
                            for nb in range(4):
                                pending.append((b, outT, qb * 4 + stq, nb))
            # final drain: attention pools are closed, so spread the o_proj
            # psums over 6 banks to decouple the matmul stream from the
            # copy+DMA latency chain
            with tc.tile_pool(name="drain_ps", bufs=6, space="PSUM") as drain_pool:
                emit_op(len(pending), split=True, pool=drain_pool)
    nc.compile()
    return nc


_GRAPH = None


def _rope_tables():
    inv_freq = 1.0 / (10000.0 ** (np.arange(0, D, 2, dtype=np.float32) / D))
    t = np.arange(S, dtype=np.float32)
    freqs = np.outer(t, inv_freq)
    emb = np.concatenate([freqs, freqs], axis=-1)  # (S, D)
    cosT = np.ascontiguousarray(np.cos(emb).T.astype(np.float32))
    sinT = np.ascontiguousarray(np.sin(emb).T.astype(np.float32))
    sinadjT = sinT.copy()
    sinadjT[0:64, :] *= -1.0  # fold rotate_half's sign into the table
    return cosT, sinadjT


def kernel(x, wq, wk, wv, wo):
    global _GRAPH, LAST_EXEC_TIME_NS, LAST_RESULTS
    import ml_dtypes

    bf16 = ml_dtypes.bfloat16
    x = np.asarray(x, dtype=np.float32)
    wq = np.asarray(wq, dtype=np.float32)
    wk = np.asarray(wk, dtype=np.float32)
    wv = np.asarray(wv, dtype=np.float32)
    wo = np.asarray(wo, dtype=np.float32)

    xT = np.ascontiguousarray(x.reshape(R, H).T)
    # xTr[rb*128+p, hc*512+c] = xT[hc*128+p, rb*512+c]
    xTr = np.ascontiguousarray(
        xT.reshape(NB_HC, 128, NB_RB, 512).transpose(2, 1, 0, 3).reshape(
            NB_RB * 128, NB_HC * 512
        )
    ).astype(bf16)
    cosT, sinadjT = _rope_tables()
    cosT = cosT.astype(bf16)
    sinadjT = sinadjT.astype(bf16)
    scale = np.float32(1.0 / np.sqrt(D))

    in_maps = []
    for c in range(NCORES):
        kv = c // HPC
        wq_c = wq[:, c * HPC * D : (c + 1) * HPC * D] * scale
        wk_c = wk[:, kv * D : (kv + 1) * D]
        wv_c = wv[:, kv * D : (kv + 1) * D]
        wqkv_c = np.concatenate([wq_c, wk_c, wv_c], axis=1, dtype=np.float32)
        # wqkvr[p, hc*512+c] = wqkv_c[hc*128+p, c]
        wqkvr = np.ascontiguousarray(
            wqkv_c.reshape(NB_HC, 128, QKV_W).transpose(1, 0, 2).reshape(
                128, NB_HC * 512
            )
        ).astype(bf16)
        wo_c = wo[c * HPC * D : (c + 1) * HPC * D, :]
        wor = np.ascontiguousarray(
            wo_c.reshape(HPC, 128, H).transpose(1, 0, 2).reshape(128, HPC * H)
        ).astype(bf16)
        in_maps.append(
            {
                "xTr": xTr,
                "wqkvr": wqkvr,
                "wor": wor,
                "cosT": cosT,
                "sinadjT": sinadjT,
            }
        )

    if _GRAPH is None:
        _GRAPH = build_graph()

    # NTFF tracing is unavailable on axon clients without antenv.axon_hooks;
    # make sure an inherited BASS_TRACE can't break execution.
    os.environ["BASS_NEVER_TRACE"] = "1"
    res = None
    for attempt in range(3):
        try:
            res = run_bass_kernel_spmd(
                _GRAPH, in_maps, core_ids=list(range(NCORES))
            )
            break
        except Exception:
            # transient axon-terminal failures (mesh desync / LoadExecutable)
            # usually clear on retry
            if attempt == 2:
                raise
            time.sleep(5.0)
    LAST_EXEC_TIME_NS = res.exec_time_ns
    LAST_RESULTS = res
    acc = np.zeros((R, H), dtype=np.float32)
    for c in range(NCORES):
        acc += np.asarray(res.results[c]["out"], dtype=np.float32)
    return acc.reshape(B, S, H)


# revision 91
# speedup vs baseline: 1.1271x; 1.0025x over previous
"""Trainium2 Bass kernel: GQA causal attention (B=2, S=2048, H=2048, 16 q-heads,
4 kv-heads, head_dim=128), tensor-parallel over 8 NeuronCores.

Sharding: 2 q-heads + their (shared) kv-head per core; wq/wk/wv column-sharded,
wo row-sharded.  Each core computes a partial o_proj output; the host sums the
8 partials (the standard TP partial-sum unshard).

All matmul operands are bf16 (PSUM accumulation stays fp32): same PE rate as
fp32r but half the DMA bytes, 2-4x DVE throughput on elementwise ops, and full
PE rate at any moving width (so causal tiles narrow to 128).

On-chip layouts are transposed (feature-on-partition) except V:
  q/k:   qkvT = w.T @ x.T            (PE, accumulate over 16 h-chunks)
  v:     natural [s, d] directly     (PE, xT chunks stationary, wv moving;
                                      no PE transposes needed)
  RoPE:  q' = q*cos + rot(q)*sin     (pure DVE: rot(q) via partition-offset
                                      muls against a sign-folded sin table)
  scoresT[k,q] = K @ Q^T             (PE; wq pre-scaled by 1/sqrt(D))
  P^T   = exp(scoresT - 40)          (ACT, fused bias; exact softmax after
                                      normalization: const cancels)
  causal mask: affine_select on P^T  (Pool/GpSimd, fill=0)
  outT  = V^T @ P^T                  (PE, PSUM-accumulated over k-chunks)
  rowsum: quad-packed ones-matmuls   (DVE pre-sums quads of P^T tiles so the
                                      PE streams 1/4 of the columns)
  outT *= bcast(1/rowsum)            (DVE recip/mul)
  out_partial = outT.T @ wo_c        (PE; outT is already the needed lhsT)

DMAs are batched (whole x row-block) to amortize the ~625ns-per-DMA HWDGE
cost; o_proj psum->sbuf drains rotate 2:1 over DVE and ACT (Pool cannot read
PSUM), and before the last batch most o_proj drips are deferred into the next
projection phase, whose DVE/ACT are otherwise idle.

HW-verified constraints honored here (the cost model does not flag them):
interleaved independent PSUM accumulation groups in one bank miscompute (so
V's four column-region groups run sequentially), and same-partition-base is
required when both DVE inputs live in SBUF (so rotate-half reads PSUM).
"""

import os
import sys
import time

import numpy as np

sys.path.insert(0, "/opt/trn_rl_repo")

from contextlib import ExitStack

import concourse.bass as bass
from concourse import bacc
import concourse.mybir as mybir
import concourse.tile as tile
from concourse.bass_utils import run_bass_kernel_spmd

F32 = mybir.dt.float32
BF16 = mybir.dt.bfloat16
AF = mybir.ActivationFunctionType
ALU = mybir.AluOpType

B, S, H = 2, 2048, 2048
NH, KVH, D = 16, 4, 128
NCORES = 8
HPC = NH // NCORES  # q heads per core = 2
R = B * S  # 4096 flattened rows
QKV_W = HPC * D + 2 * D  # 512 = [q0|q1|k|v] columns per core
NB_RB = R // 512  # 8 row-blocks of 512
NB_HC = H // 128  # 16 contraction chunks
SB = S // 512  # 4 q-blocks per batch
SC = S // 128  # 16 k-chunks per batch
EXP_BIAS = -40.0

LAST_EXEC_TIME_NS = None
LAST_RESULTS = None


def build_graph(reps=1):
    nc = bacc.Bacc(
        "TRN2", target_bir_lowering=False, debug=False, num_devices=NCORES
    )
    # host-prepared layouts (see kernel()): xTr[rb*128+p, hc*512+c] =
    # x.T[hc*128+p, rb*512+c]; wqkvr[p, hc*512+c] = wqkv[hc*128+p, c];
    # wor[p, h*2048+c] = wo[h*128+p, c].
    xTr = nc.dram_tensor("xTr", [NB_RB * 128, NB_HC * 512], BF16, kind="ExternalInput").ap()
    wqkvr = nc.dram_tensor("wqkvr", [128, NB_HC * 512], BF16, kind="ExternalInput").ap()
    wor = nc.dram_tensor("wor", [128, HPC * H], BF16, kind="ExternalInput").ap()
    cosT = nc.dram_tensor("cosT", [D, S], BF16, kind="ExternalInput").ap()
    sinadjT = nc.dram_tensor("sinadjT", [D, S], BF16, kind="ExternalInput").ap()
    out = nc.dram_tensor("out", [R, H], BF16, kind="ExternalOutput").ap()

    with tile.TileContext(nc) as tc, ExitStack() as ctx:
        # ---- persistent SBUF ----
        const_pool = ctx.enter_context(tc.tile_pool(name="const", bufs=1))
        w_sb = const_pool.tile([128, NB_HC * 512], BF16)
        wo_sb = const_pool.tile([128, HPC * H], BF16)
        cos_sb = const_pool.tile([128, S], BF16)
        sinadj_sb = const_pool.tile([128, S], BF16)
        ones_sb = const_pool.tile([128, 128], BF16)  # rowsum lhsT / bcast
        expb_sb = const_pool.tile([128, 1], F32)  # exp bias (per-partition)
        # qk/v live in per-row-block tiles: tile-granular dependency tracking
        # would otherwise serialize attention's first reads behind the LAST
        # row-block's RoPE/copy on the DVE queue.
        qk_sb = {
            (cg, rb): const_pool.tile([128, 512], BF16, name=f"qk{cg}_{rb}")
            for cg in range(3)
            for rb in range(NB_RB)
        }
        v_sb = {
            rb: const_pool.tile([128, 512], BF16, name=f"v{rb}")
            for rb in range(NB_RB)
        }
        scr_sb = const_pool.tile([128, 1], F32)

        nc.gpsimd.memset(ones_sb[:], 1.0)
        nc.gpsimd.memset(expb_sb[:], EXP_BIAS)
        # touch Exp once so the ACT table load happens while ACT is idle,
        # not in front of the first real softmax tile
        nc.scalar.activation(scr_sb[:], expb_sb[:], AF.Exp, bias=0.0, scale=1.0)

        outT_pool = ctx.enter_context(tc.tile_pool(name="outT", bufs=2))
        ms_ps_pool = ctx.enter_context(tc.tile_pool(name="ms_ps", bufs=2, space="PSUM"))
        osb_pool = ctx.enter_context(tc.tile_pool(name="osb", bufs=6))
        xt_pool = ctx.enter_context(tc.tile_pool(name="xt", bufs=2))
        # scratch SBUF pools are persistent: per-phase pools would reuse the
        # same addresses and stall each phase's first ops on the previous
        # phase's last frees
        rtmp_pool = ctx.enter_context(tc.tile_pool(name="rtmp", bufs=8))
        pt_pool = ctx.enter_context(tc.tile_pool(name="pt", bufs=28))
        s2_pool = ctx.enter_context(tc.tile_pool(name="s2", bufs=8))
        s4_pool = ctx.enter_context(tc.tile_pool(name="s4", bufs=10))
        rr_sb_pool = ctx.enter_context(tc.tile_pool(name="rr_sb", bufs=2))

        # ---- o_proj drip FIFO: one (row-block, nb) pair per emission so the
        # in-order PE queue always has other matmuls between an o_proj pair
        # and its psum-slot dependency (the psum->sbuf copy). Output rows are
        # staged in a [128, 2048] row buffer and DMA'd once per row-block.
        pending = []
        ncopy = [0]
        osb_rows = {}

        def emit_op(nmax, split=False, pool=None, defer_below=0):
            for _ in range(nmax):
                if len(pending) <= defer_below:
                    return
                ob, oT, st, nb = pending.pop(0)
                op_ps = (pool or ms_ps_pool).tile(
                    [128, 512], F32, tag="ms", name="op_ps"
                )
                for h in range(HPC):
                    nc.tensor.matmul(
                        op_ps[:],
                        oT[:, h * S + st * 128 : h * S + (st + 1) * 128],
                        wo_sb[:, h * H + nb * 512 : h * H + (nb + 1) * 512],
                        start=(h == 0),
                        stop=(h == HPC - 1),
                    )
                r0 = ob * S + st * 128
                if split:
                    # tail flush: copies alternate engines per tile and land
                    # in a row buffer, so the trailing out-writes are 4 wide
                    # DMAs instead of 16 HWDGE dispatches
                    key = (ob, st)
                    if nb == 0:
                        osb_rows[key] = osb_pool.tile(
                            [128, H], BF16, tag="orow", name="orow"
                        )
                    orow = osb_rows[key]
                    dst = orow[:, nb * 512 : (nb + 1) * 512]
                    if ncopy[0] % 2 == 0:
                        nc.vector.tensor_copy(dst, op_ps[:])
                    else:
                        nc.scalar.copy(dst, op_ps[:])
                    if nb == 3:
                        del osb_rows[key]
                        nc.sync.dma_start(out[r0 : r0 + 128, :], orow[:])
                else:
                    osb = osb_pool.tile([128, 512], BF16, tag="osb", name="osb")
                    # psum drains rotate 2:1 over DVE and ACT (ACT also
                    # carries the softmax exps; Pool cannot read PSUM)
                    if ncopy[0] % 3 == 1:
                        nc.scalar.copy(osb[:], op_ps[:])
                    else:
                        nc.vector.tensor_copy(osb[:], op_ps[:])
                    nc.sync.dma_start(
                        out[r0 : r0 + 128, nb * 512 : (nb + 1) * 512], osb[:]
                    )
                ncopy[0] += 1

        xts = {}

        def fetch_xt(rb, granularity):
            t = xt_pool.tile([128, NB_HC * 512], BF16, tag="xt")
            step = NB_HC // granularity
            for g in range(granularity):
                sl = slice(g * step * 512, (g + 1) * step * 512)
                nc.sync.dma_start(t[:, sl], xTr[rb * 128 : (rb + 1) * 128, sl])
            xts[rb] = t

        for _rep in range(reps):
            for b in range(B):
                # ---- phase 1: qkvT matmuls + RoPE (q/k) + natural V ----
                with (
                    tc.tile_pool(name="qkv_ps", bufs=5, space="PSUM") as qkv_ps_pool,
                ):
                    for rbl in range(SB):
                        rb = b * SB + rbl
                        if rb == 0:
                            # startup: stream w and x at fine granularity so
                            # the first matmuls' deps land early; rope tables
                            # right after the first chunks.
                            t = xt_pool.tile([128, NB_HC * 512], BF16, tag="xt")
                            xts[0] = t
                            for lo, hi in [(0, 1), (1, 2), (2, 4), (4, 6),
                                           (6, 8), (8, 10), (10, 12), (12, 14),
                                           (14, 16)]:
                                sl = slice(lo * 512, hi * 512)
                                nc.sync.dma_start(w_sb[:, sl], wqkvr[:, sl])
                                nc.sync.dma_start(t[:, sl], xTr[0:128, sl])
                        xt = xts.pop(rb)
                        if rbl + 1 < SB:
                            fetch_xt(rb + 1, 2)
                        elif b + 1 < B:
                            fetch_xt((b + 1) * SB, 2)
                        if rb == 0:
                            # rope tables + wo after the next row-block's x:
                            # they are only needed once DMA has caught up
                            nc.sync.dma_start(cos_sb[:], cosT)
                            nc.sync.dma_start(sinadj_sb[:], sinadjT)
                            nc.sync.dma_start(wo_sb[:], wor)
                        # Each projection runs as ONE sequential accumulation
                        # group (k first, then q0/q1, then V's four column
                        # regions): each group's rope/drain starts while the
                        # next group is still on the PE, so by row-block end
                        # no psum-read backlog remains to stall the phase
                        # handoff. (Interleaving independent start/stop
                        # groups in one PSUM bank also miscomputes on HW.)
                        cs = cos_sb[:, rbl * 512 : (rbl + 1) * 512]
                        sn_lo = sinadj_sb[0:64, rbl * 512 : (rbl + 1) * 512]
                        sn_hi = sinadj_sb[64:128, rbl * 512 : (rbl + 1) * 512]

                        def rope_drain(ps, dst):
                            # drain the bank with one bf16 ACT copy, then
                            # rope on DVE: the rotate-half reads come
                            # partition-offset straight from PSUM (the
                            # equal-base rule only binds when both inputs are
                            # SBUF); psum reads go first so the bank frees
                            # as early as possible
                            qraw = rtmp_pool.tile(
                                [128, 512], BF16, tag="rtmp", name="qraw"
                            )
                            nc.scalar.copy(qraw[:], ps[:])
                            t1 = rtmp_pool.tile([128, 512], BF16, tag="rtmp", name="t1")
                            t2 = rtmp_pool.tile([128, 512], BF16, tag="rtmp", name="t2")
                            nc.vector.tensor_mul(t2[0:64, :], ps[64:128, :], sn_lo)
                            nc.vector.tensor_mul(t2[64:128, :], ps[0:64, :], sn_hi)
                            nc.vector.tensor_mul(t1[:], qraw[:], cs)
                            nc.vector.tensor_add(dst, t1[:], t2[:])

                        for cg, dst_key in ((2, 2), (0, 0), (1, 1)):
                            g_ps = qkv_ps_pool.tile(
                                [128, 512], F32, tag="qkvps", name=f"g{cg}"
                            )
                            for hc in range(NB_HC):
                                nc.tensor.matmul(
                                    g_ps[:],
                                    w_sb[:, hc * 512 + cg * 128 : hc * 512 + (cg + 1) * 128],
                                    xt[:, hc * 512 : (hc + 1) * 512],
                                    start=(hc == 0),
                                    stop=(hc == NB_HC - 1),
                                )
                                if hc % 4 == 1:
                                    emit_op(1)
                            rope_drain(g_ps, qk_sb[(dst_key, rb)][:])
                        v_ps = qkv_ps_pool.tile([128, 512], F32, tag="qkvps", name="v")
                        for rc in range(4):
                            for hc in range(NB_HC):
                                nc.tensor.matmul(
                                    v_ps[:, rc * 128 : (rc + 1) * 128],
                                    xt[:, hc * 512 + rc * 128 : hc * 512 + (rc + 1) * 128],
                                    w_sb[:, hc * 512 + 384 : hc * 512 + 512],
                                    start=(hc == 0),
                                    stop=(hc == NB_HC - 1),
                                )
                            emit_op(1)
                        nc.scalar.copy(v_sb[rb][:], v_ps[:])

                # ---- phase 2: attention for batch b ----
                # pool-open order controls bank placement: rs (written last)
                # takes the banks freed last by phase 1; st (needed first)
                # lands on the earliest-freed/spare banks
                with (
                    tc.tile_pool(name="rs_ps", bufs=2, space="PSUM") as rs_ps_pool,
                    tc.tile_pool(name="ot_ps", bufs=2, space="PSUM") as ot_ps_pool,
                    tc.tile_pool(name="st_ps", bufs=2, space="PSUM") as st_ps_pool,
                ):
                    PD = 5  # per-head pipeline depth
                    outT = outT_pool.tile([128, HPC * S], BF16)
                    for qb in range(SB):
                        o_ps, r_ps = {}, {}
                        for h in range(HPC):
                            o_ps[h] = ot_ps_pool.tile(
                                [128, 512], F32, tag="ot", name=f"ot{h}"
                            )
                            r_ps[h] = rs_ps_pool.tile(
                                [128, 512], F32, tag="rs", name=f"rs{h}"
                            )
                        nj = 4 * qb + 4
                        pd = 3 if (b == B - 1 and qb == SB - 1) else PD
                        # before the last batch, defer most o_proj drips into
                        # the next phase 1, whose DVE/ACT are mostly idle —
                        # in attention both engines are near-saturated
                        dfb = 20 if b < B - 1 else 0
                        pts = {}
                        s2s = {}
                        s4s = {}
                        diag = {}
                        for jj in range(nj + pd):
                            emit_op(2 if jj < 2 else 1, defer_below=dfb)
                            if jj < nj:
                                j = jj
                                r = j - 4 * qb  # diagonal band index
                                qoff = 128 * r if r > 0 else 0
                                W = 512 - qoff
                                for h in range(HPC):
                                    s_ps = st_ps_pool.tile([128, 512], F32)
                                    nc.tensor.matmul(
                                        s_ps[:, qoff:512],
                                        qk_sb[(2, b * SB + j // 4)][
                                            :, (j % 4) * 128 : (j % 4 + 1) * 128
                                        ],
                                        qk_sb[(h, b * SB + qb)][:, qoff:512],
                                        start=True,
                                        stop=True,
                                    )
                                    pt = pt_pool.tile([128, 512], BF16)
                                    nc.scalar.activation(
                                        pt[:, qoff:512],
                                        s_ps[:, qoff:512],
                                        AF.Exp,
                                        bias=expb_sb[:],
                                        scale=1.0,
                                    )
                                    if r >= 0:
                                        # zero where k > q inside the 128-wide
                                        # diagonal ramp
                                        nc.gpsimd.affine_select(
                                            out=pt[:, qoff : qoff + 128],
                                            in_=pt[:, qoff : qoff + 128],
                                            pattern=[[1, 128]],
                                            compare_op=ALU.is_ge,
                                            fill=0.0,
                                            base=0,
                                            channel_multiplier=-1,
                                        )
                                    pts[(h, j)] = (pt, qoff, W)
                                    # rowsum packing on DVE (all-bf16 = fast):
                                    padd = nc.vector.tensor_add
                                    pcopy = nc.vector.tensor_copy
                                    if j < 4 * qb:
                                        if j % 2 == 1:
                                            s2 = s2_pool.tile([128, 512], BF16, tag="s2")
                                            padd(s2[:], pts[(h, j - 1)][0][:], pt[:])
                                            s2s[(h, j // 2)] = s2
                                        if j % 4 == 3:
                                            s4 = s4_pool.tile([128, 512], BF16, tag="s4")
                                            padd(
                                                s4[:],
                                                s2s.pop((h, j // 2 - 1))[:],
                                                s2s.pop((h, j // 2))[:],
                                            )
                                            s4s[(h, j // 4)] = s4
                                    elif r == 1:
                                        pt0 = pts[(h, 4 * qb)][0]
                                        sa = s4_pool.tile([128, 512], BF16, tag="s4")
                                        pcopy(sa[:, 0:128], pt0[:, 0:128])
                                        padd(
                                            sa[:, 128:512],
                                            pt0[:, 128:512],
                                            pt[:, 128:512],
                                        )
                                        diag[(h, 0)] = sa
                                    elif r == 3:
                                        pt2 = pts[(h, 4 * qb + 2)][0]
                                        sb_ = s4_pool.tile([128, 512], BF16, tag="s4")
                                        pcopy(sb_[:, 256:384], pt2[:, 256:384])
                                        padd(
                                            sb_[:, 384:512],
                                            pt2[:, 384:512],
                                            pt[:, 384:512],
                                        )
                                        diag[(h, 1)] = sb_
                            if jj >= pd:
                                j2 = jj - pd
                                for h in range(HPC):
                                    pt2, qoff2, W2 = pts.pop((h, j2))
                                    if j2 < 4 * qb:
                                        if j2 % 4 == 3:
                                            s4c = s4s.pop((h, j2 // 4))
                                            nc.tensor.matmul(
                                                r_ps[h][:],
                                                ones_sb[:],
                                                s4c[:],
                                                start=(j2 == 3),
                                                stop=False,
                                                skip_group_check=True,
                                            )
                                    elif j2 == 4 * qb + 1:
                                        nc.tensor.matmul(
                                            r_ps[h][:],
                                            ones_sb[:],
                                            diag[(h, 0)][:],
                                            start=(qb == 0),
                                            stop=False,
                                            skip_group_check=True,
                                        )
                                    elif j2 == 4 * qb + 3:
                                        nc.tensor.matmul(
                                            r_ps[h][:, 256:512],
                                            ones_sb[:],
                                            diag[(h, 1)][:, 256:512],
                                            start=False,
                                            stop=True,
                                            skip_group_check=True,
                                        )
                                    nc.tensor.matmul(
                                        o_ps[h][:, qoff2:512],
                                        v_sb[b * SB + j2 // 4][
                                            :, (j2 % 4) * 128 : (j2 % 4 + 1) * 128
                                        ],
                                        pt2[:, qoff2:512],
                                        start=(j2 == 0),
                                        stop=(j2 == nj - 1),
                                        skip_group_check=True,
                                    )
                            emit_op(1, defer_below=dfb)
                        for h in range(HPC):
                            rr = rr_sb_pool.tile([128, 512], F32, tag="rr")
                            nc.vector.reciprocal(rr[:], r_ps[h][:])
                            nc.vector.tensor_mul(
                                outT[:, h * S + qb * 512 : h * S + (qb + 1) * 512],
                                o_ps[h][:],
                                rr[:],
                            )
                        for stq in range(4):
                            for nb in range(4):
                                pending.append((b, outT, qb * 4 + stq, nb))
            # final drain: attention pools are closed, so spread the o_proj
            # psums over 6 banks to decouple the matmul stream from the
            # copy+DMA latency chain
            with tc.tile_pool(name="drain_ps", bufs=6, space="PSUM") as drain_pool:
                emit_op(len(pending), split=True, pool=drain_pool)
    nc.compile()
    return nc


_GRAPH = None


def _rope_tables():
    inv_freq = 1.0 / (10000.0 ** (np.arange(0, D, 2, dtype=np.float32) / D))
    t = np.arange(S, dtype=np.float32)
    freqs = np.outer(t, inv_freq)
    emb = np.concatenate([freqs, freqs], axis=-1)  # (S, D)
    cosT = np.ascontiguousarray(np.cos(emb).T.astype(np.float32))
    sinT = np.ascontiguousarray(np.sin(emb).T.astype(np.float32))
    sinadjT = sinT.copy()
    sinadjT[0:64, :] *= -1.0  # fold rotate_half's sign into the table
    return cosT, sinadjT


def kernel(x, wq, wk, wv, wo):
    global _GRAPH, LAST_EXEC_TIME_NS, LAST_RESULTS
    import ml_dtypes

    bf16 = ml_dtypes.bfloat16
    x = np.asarray(x, dtype=np.float32)
    wq = np.asarray(wq, dtype=np.float32)
    wk = np.asarray(wk, dtype=np.float32)
    wv = np.asarray(wv, dtype=np.float32)
    wo = np.asarray(wo, dtype=np.float32)

    xT = np.ascontiguousarray(x.reshape(R, H).T)
    # xTr[rb*128+p, hc*512+c] = xT[hc*128+p, rb*512+c]
    xTr = np.ascontiguousarray(
        xT.reshape(NB_HC, 128, NB_RB, 512).transpose(2, 1, 0, 3).reshape(
            NB_RB * 128, NB_HC * 512
        )
    ).astype(bf16)
    cosT, sinadjT = _rope_tables()
    cosT = cosT.astype(bf16)
    sinadjT = sinadjT.astype(bf16)
    scale = np.float32(1.0 / np.sqrt(D))

    in_maps = []
    for c in range(NCORES):
        kv = c // HPC
        wq_c = wq[:, c * HPC * D : (c + 1) * HPC * D] * scale
        wk_c = wk[:, kv * D : (kv + 1) * D]
        wv_c = wv[:, kv * D : (kv + 1) * D]
        wqkv_c = np.concatenate([wq_c, wk_c, wv_c], axis=1, dtype=np.float32)
        # wqkvr[p, hc*512+c] = wqkv_c[hc*128+p, c]
        wqkvr = np.ascontiguousarray(
            wqkv_c.reshape(NB_HC, 128, QKV_W).transpose(1, 0, 2).reshape(
                128, NB_HC * 512
            )
        ).astype(bf16)
        wo_c = wo[c * HPC * D : (c + 1) * HPC * D, :]
        wor = np.ascontiguousarray(
            wo_c.reshape(HPC, 128, H).transpose(1, 0, 2).reshape(128, HPC * H)
        ).astype(bf16)
        in_maps.append(
            {
                "xTr": xTr,
                "wqkvr": wqkvr,
                "wor": wor,
                "cosT": cosT,
                "sinadjT": sinadjT,
            }
        )

    if _GRAPH is None:
        _GRAPH = build_graph()

    # NTFF tracing is unavailable on axon clients without antenv.axon_hooks;
    # make sure an inherited BASS_TRACE can't break execution.
    os.environ["BASS_NEVER_TRACE"] = "1"
    res = None
    for attempt in range(3):
        try:
            res = run_bass_kernel_spmd(
                _GRAPH, in_maps, core_ids=list(range(NCORES))
            )
            break
        except Exception:
            # transient axon-terminal failures (mesh desync / LoadExecutable)
            # usually clear on retry
            if attempt == 2:
                raise
            time.sleep(5.0)
    LAST_EXEC_TIME_NS = res.exec_time_ns
    LAST_RESULTS = res
    acc = np.zeros((R, H), dtype=np.float32)
    for c in range(NCORES):
        acc += np.asarray(res.results[c]["out"], dtype=np.float32)
    return acc.reshape(B, S, H)


# revision 92
# speedup vs baseline: 1.1471x; 1.0178x over previous
"""Trainium2 Bass kernel: GQA causal attention (B=2, S=2048, H=2048, 16 q-heads,
4 kv-heads, head_dim=128), tensor-parallel over 8 NeuronCores.

Sharding: 2 q-heads + their (shared) kv-head per core; wq/wk/wv column-sharded,
wo row-sharded.  Each core computes a partial o_proj output; the host sums the
8 partials (the standard TP partial-sum unshard).

All matmul operands are bf16 (PSUM accumulation stays fp32): same PE rate as
fp32r but half the DMA bytes, 2-4x DVE throughput on elementwise ops, and full
PE rate at any moving width (so causal tiles narrow to 128).

On-chip layouts are transposed (feature-on-partition) except V:
  q/k:   qkvT = w.T @ x.T            (PE, accumulate over 16 h-chunks)
  v:     natural [s, d] directly     (PE, xT chunks stationary, wv moving;
                                      no PE transposes needed)
  RoPE:  q' = q*cos + rot(q)*sin     (pure DVE: rot(q) via partition-offset
                                      muls against a sign-folded sin table)
  scoresT[k,q] = K @ Q^T             (PE; wq pre-scaled by 1/sqrt(D))
  P^T   = exp(scoresT - 40)          (ACT, fused bias; exact softmax after
                                      normalization: const cancels)
  causal mask: affine_select on P^T  (Pool/GpSimd, fill=0)
  outT  = V^T @ P^T                  (PE, PSUM-accumulated over k-chunks)
  rowsum: quad-packed ones-matmuls   (DVE pre-sums quads of P^T tiles so the
                                      PE streams 1/4 of the columns)
  outT *= bcast(1/rowsum)            (DVE recip/mul)
  out_partial = outT.T @ wo_c        (PE; outT is already the needed lhsT)

DMAs are batched (whole x row-block / whole output row) to amortize the
~625ns-per-DMA HWDGE cost; o_proj psum->sbuf copies rotate across DVE, Pool
and ACT so no single engine becomes co-critical with the PE.
"""

import os
import sys
import time

import numpy as np

sys.path.insert(0, "/opt/trn_rl_repo")

from contextlib import ExitStack

import concourse.bass as bass
from concourse import bacc
import concourse.mybir as mybir
import concourse.tile as tile
from concourse.bass_utils import run_bass_kernel_spmd

F32 = mybir.dt.float32
BF16 = mybir.dt.bfloat16
AF = mybir.ActivationFunctionType
ALU = mybir.AluOpType

B, S, H = 2, 2048, 2048
NH, KVH, D = 16, 4, 128
NCORES = 8
HPC = NH // NCORES  # q heads per core = 2
R = B * S  # 4096 flattened rows
QKV_W = HPC * D + 2 * D  # 512 = [q0|q1|k|v] columns per core
NB_RB = R // 512  # 8 row-blocks of 512
NB_HC = H // 128  # 16 contraction chunks
SB = S // 512  # 4 q-blocks per batch
SC = S // 128  # 16 k-chunks per batch
EXP_BIAS = -40.0

LAST_EXEC_TIME_NS = None
LAST_RESULTS = None


def build_graph(reps=1):
    nc = bacc.Bacc(
        "TRN2", target_bir_lowering=False, debug=False, num_devices=NCORES
    )
    # host-prepared layouts (see kernel()): xTr[rb*128+p, hc*512+c] =
    # x.T[hc*128+p, rb*512+c]; wqkvr[p, hc*512+c] = wqkv[hc*128+p, c];
    # wor[p, h*2048+c] = wo[h*128+p, c].
    xTr = nc.dram_tensor("xTr", [NB_RB * 128, NB_HC * 512], BF16, kind="ExternalInput").ap()
    # kT-dedup: each core's xTr is permuted so its own batch comes first
    # (even cores: batch 0, odd: batch 1 — the pair shares one kv head);
    # each core projects+ropes kT only for that local batch, the pair
    # AllGathers the halves, and the partner half is reconstructed exactly
    # as (slot0+slot1)-local in fp32. The host unpermutes the output rows.
    kvloc = nc.dram_tensor("kvloc", [128, S], BF16, kind="Internal").ap()
    kvglob = nc.dram_tensor("kvglob", [256, S], BF16, kind="Internal").ap()
    wqkvr = nc.dram_tensor("wqkvr", [128, NB_HC * 512], BF16, kind="ExternalInput").ap()
    wor = nc.dram_tensor("wor", [128, HPC * H], BF16, kind="ExternalInput").ap()
    cosT = nc.dram_tensor("cosT", [D, S], BF16, kind="ExternalInput").ap()
    sinadjT = nc.dram_tensor("sinadjT", [D, S], BF16, kind="ExternalInput").ap()
    out = nc.dram_tensor("out", [R, H], BF16, kind="ExternalOutput").ap()

    with tile.TileContext(nc) as tc, ExitStack() as ctx:
        # ---- persistent SBUF ----
        const_pool = ctx.enter_context(tc.tile_pool(name="const", bufs=1))
        w_sb = const_pool.tile([128, NB_HC * 512], BF16)
        wo_sb = const_pool.tile([128, HPC * H], BF16)
        cos_sb = const_pool.tile([128, S], BF16)
        sinadj_sb = const_pool.tile([128, S], BF16)
        ones_sb = const_pool.tile([128, 128], BF16)  # rowsum lhsT / bcast
        expb_sb = const_pool.tile([128, 1], F32)  # exp bias (per-partition)
        # qk/v live in per-row-block tiles: tile-granular dependency tracking
        # would otherwise serialize attention's first reads behind the LAST
        # row-block's RoPE/copy on the DVE queue.
        qk_sb = {
            (cg, rb): const_pool.tile([128, 512], BF16, name=f"qk{cg}_{rb}")
            for cg in range(2)
            for rb in range(NB_RB)
        }
        kT_sb = const_pool.tile([128, R], BF16)  # gathered roped kT, both batches
        vall_sb = const_pool.tile([128, R], BF16)  # gathered natural V
        g0_sb = const_pool.tile([128, S], BF16)
        g1_sb = const_pool.tile([128, S], BF16)
        scr_sb = const_pool.tile([128, 1], F32)

        nc.gpsimd.memset(ones_sb[:], 1.0)
        nc.gpsimd.memset(expb_sb[:], EXP_BIAS)
        # touch Exp once so the ACT table load happens while ACT is idle,
        # not in front of the first real softmax tile
        nc.scalar.activation(scr_sb[:], expb_sb[:], AF.Exp, bias=0.0, scale=1.0)

        outT_pool = ctx.enter_context(tc.tile_pool(name="outT", bufs=2))
        ms_ps_pool = ctx.enter_context(tc.tile_pool(name="ms_ps", bufs=2, space="PSUM"))
        osb_pool = ctx.enter_context(tc.tile_pool(name="osb", bufs=6))
        xt_pool = ctx.enter_context(tc.tile_pool(name="xt", bufs=2))
        # scratch SBUF pools are persistent: per-phase pools would reuse the
        # same addresses and stall each phase's first ops on the previous
        # phase's last frees
        rtmp_pool = ctx.enter_context(tc.tile_pool(name="rtmp", bufs=8))
        pt_pool = ctx.enter_context(tc.tile_pool(name="pt", bufs=14))
        s2_pool = ctx.enter_context(tc.tile_pool(name="s2", bufs=6))
        s4_pool = ctx.enter_context(tc.tile_pool(name="s4", bufs=8))
        rr_sb_pool = ctx.enter_context(tc.tile_pool(name="rr_sb", bufs=2))

        # ---- o_proj drip FIFO: one (row-block, nb) pair per emission so the
        # in-order PE queue always has other matmuls between an o_proj pair
        # and its psum-slot dependency (the psum->sbuf copy). Output rows are
        # staged in a [128, 2048] row buffer and DMA'd once per row-block.
        pending = []
        ncopy = [0]
        outT_by_b = {}

        def emit_op(nmax, split=False, pool=None, defer_below=0):
            for _ in range(nmax):
                if len(pending) <= defer_below:
                    return
                ob, oT, st, nb = pending.pop(0)
                op_ps = (pool or ms_ps_pool).tile(
                    [128, 512], F32, tag="ms", name="op_ps"
                )
                for h in range(HPC):
                    nc.tensor.matmul(
                        op_ps[:],
                        oT[:, h * S + st * 128 : h * S + (st + 1) * 128],
                        wo_sb[:, h * H + nb * 512 : h * H + (nb + 1) * 512],
                        start=(h == 0),
                        stop=(h == HPC - 1),
                    )
                osb = osb_pool.tile([128, 512], BF16, tag="osb", name="osb")
                if split:  # tail flush: alternate engines per tile
                    if ncopy[0] % 2 == 0:
                        nc.vector.tensor_copy(osb[:], op_ps[:])
                    else:
                        nc.scalar.copy(osb[:], op_ps[:])
                else:
                    # psum drains rotate 2:1 over DVE and ACT (ACT also
                    # carries the softmax exps; Pool cannot read PSUM)
                    if ncopy[0] % 3 == 1:
                        nc.scalar.copy(osb[:], op_ps[:])
                    else:
                        nc.vector.tensor_copy(osb[:], op_ps[:])
                ncopy[0] += 1
                r0 = ob * S + st * 128
                nc.sync.dma_start(
                    out[r0 : r0 + 128, nb * 512 : (nb + 1) * 512], osb[:]
                )

        xts = {}

        def fetch(src, row0, key, granularity=2):
            t = xt_pool.tile([128, NB_HC * 512], BF16, tag="xt")
            step = NB_HC // granularity
            for g in range(granularity):
                sl = slice(g * step * 512, (g + 1) * step * 512)
                nc.sync.dma_start(t[:, sl], src[row0 : row0 + 128, sl])
            xts[key] = t

        def rope(ps, qraw, dst, pos, out_pool):
            # RoPE: q' = q*cos + rot(q)*sin; rotate-half reads come
            # partition-offset straight from PSUM (the equal-base rule only
            # binds when both inputs are SBUF); psum reads go first so the
            # bank frees as early as possible. sinadj has rotate_half's sign
            # folded in: sinadj[0:64] = -sin[0:64], sinadj[64:128] = +sin.
            cs = cos_sb[:, pos * 512 : (pos + 1) * 512]
            sn_lo = sinadj_sb[0:64, pos * 512 : (pos + 1) * 512]
            sn_hi = sinadj_sb[64:128, pos * 512 : (pos + 1) * 512]
            t1 = rtmp_pool.tile([128, 512], BF16, tag="rtmp")
            t2 = rtmp_pool.tile([128, 512], BF16, tag="rtmp")
            nc.vector.tensor_mul(t2[0:64, :], ps[64:128, :], sn_lo)
            nc.vector.tensor_mul(t2[64:128, :], ps[0:64, :], sn_hi)
            nc.vector.tensor_mul(t1[:], qraw[:], cs)
            nc.vector.tensor_add(dst, t1[:], t2[:])

        for _rep in range(reps):
            for b in range(B):
                # ---- phase 1: q + V (+ local kT for b==0) projections and
                # RoPE; the pair AllGather of roped kT halves runs under the
                # local-batch attention ----
                with (
                    tc.tile_pool(name="q_ps", bufs=5, space="PSUM") as q_ps_pool,
                ):
                    if b == 1:
                        # exchange the roped local-kT halves within the pair;
                        # emitted here so no queue parks on it during the
                        # local-batch attention
                        nc.gpsimd.collective_compute(
                            "AllGather", ALU.bypass,
                            [[2 * p, 2 * p + 1] for p in range(NCORES // 2)],
                            ins=[kvloc], outs=[kvglob],
                        )
                    for rbl in range(SB):
                        rb = b * SB + rbl
                        if rb == 0:
                            # startup: stream w and x at fine granularity so
                            # the first matmuls' deps land early
                            t = xt_pool.tile([128, NB_HC * 512], BF16, tag="xt")
                            xts[("q", 0)] = t
                            for lo, hi in [(0, 1), (1, 2), (2, 4), (4, 6),
                                           (6, 8), (8, 10), (10, 12),
                                           (12, 14), (14, 16)]:
                                sl = slice(lo * 512, hi * 512)
                                nc.sync.dma_start(w_sb[:, sl], wqkvr[:, sl])
                                nc.sync.dma_start(t[:, sl], xTr[0:128, sl])
                        xt = xts.pop(("q", rb))
                        if rb + 1 < NB_RB:
                            fetch(xTr, (rb + 1) * 128, ("q", rb + 1))
                        if rb == 0:
                            nc.sync.dma_start(cos_sb[:], cosT)
                            nc.sync.dma_start(sinadj_sb[:], sinadjT)
                            nc.sync.dma_start(wo_sb[:], wor)
                        q0_ps = q_ps_pool.tile([128, 512], F32, tag="qps", name="q0")
                        q1_ps = q_ps_pool.tile([128, 512], F32, tag="qps", name="q1")
                        v_ps = q_ps_pool.tile([128, 512], F32, tag="qps", name="v")
                        q_list = [q0_ps, q1_ps]
                        if b == 0:
                            k_ps = q_ps_pool.tile([128, 512], F32, tag="qps", name="k")
                            for hc in range(NB_HC):
                                nc.tensor.matmul(
                                    k_ps[:],
                                    w_sb[:, hc * 512 + 256 : hc * 512 + 384],
                                    xt[:, hc * 512 : (hc + 1) * 512],
                                    start=(hc == 0),
                                    stop=(hc == NB_HC - 1),
                                )
                        for hc in range(NB_HC):
                            xsl = xt[:, hc * 512 : (hc + 1) * 512]
                            for cg in range(2):
                                nc.tensor.matmul(
                                    q_list[cg][:],
                                    w_sb[:, hc * 512 + cg * 128 : hc * 512 + (cg + 1) * 128],
                                    xsl,
                                    start=(hc == 0),
                                    stop=(hc == NB_HC - 1),
                                )
                            emit_op(1)
                        # v: one accumulation group at a time — interleaving
                        # independent start/stop groups in different column
                        # regions of one PSUM bank miscomputes on HW
                        for rc in range(4):
                            for hc in range(NB_HC):
                                nc.tensor.matmul(
                                    v_ps[:, rc * 128 : (rc + 1) * 128],
                                    xt[:, hc * 512 + rc * 128 : hc * 512 + (rc + 1) * 128],
                                    w_sb[:, hc * 512 + 384 : hc * 512 + 512],
                                    start=(hc == 0),
                                    stop=(hc == NB_HC - 1),
                                )
                            emit_op(1)
                        # Drain each psum bank with a single bf16 copy on the
                        # phase-1-idle ACT engine so banks free fast, then
                        # rope from the SBUF copies at bf16 throughput.
                        nc.scalar.copy(
                            vall_sb[:, rb * 512 : (rb + 1) * 512], v_ps[:]
                        )
                        raws = {}
                        if b == 0:
                            kraw = rtmp_pool.tile([128, 512], BF16, tag="rtmp")
                            nc.scalar.copy(kraw[:], k_ps[:])
                            rope(
                                k_ps, kraw,
                                kT_sb[:, rbl * 512 : (rbl + 1) * 512],
                                rbl, rtmp_pool,
                            )
                            nc.scalar.dma_start(
                                kvloc[:, rbl * 512 : (rbl + 1) * 512],
                                kT_sb[:, rbl * 512 : (rbl + 1) * 512],
                            )
                        for cg in range(2):
                            qraw = rtmp_pool.tile([128, 512], BF16, tag="rtmp")
                            nc.scalar.copy(qraw[:], q_list[cg][:])
                            raws[cg] = qraw
                        for cg in range(2):
                            rope(
                                q_list[cg], raws[cg], qk_sb[(cg, rb)][:],
                                rbl, rtmp_pool,
                            )
                    if b == 1:
                        # reconstruct the partner's roped kT exactly:
                        # bf16+bf16 in fp32 is exact, so (g0+g1)-local is
                        # bit-exact the partner half
                        gate = outT_by_b[0]
                        nc.vector.tensor_copy(
                            g0_sb[0:1, 0:1], gate[0:1, HPC * S - 1 : HPC * S]
                        )
                        nc.vector.tensor_copy(
                            g1_sb[0:1, 0:1], gate[0:1, HPC * S - 1 : HPC * S]
                        )
                        nc.scalar.dma_start(g0_sb[:], kvglob[0:128, :])
                        nc.scalar.dma_start(g1_sb[:], kvglob[128:256, :])
                        for tb in range(SB):
                            sl = slice(tb * 512, (tb + 1) * 512)
                            gs = rtmp_pool.tile([128, 512], F32, tag="gsum", name="gs")
                            nc.vector.tensor_add(gs[:], g0_sb[:, sl], g1_sb[:, sl])
                            nc.vector.tensor_sub(
                                kT_sb[:, S + tb * 512 : S + (tb + 1) * 512],
                                gs[:],
                                kT_sb[:, sl],
                            )

                # ---- phase 2: attention for batch b ----
                # pool-open order controls bank placement: rs (written last)
                # takes the banks freed last by phase 1; st (needed first)
                # lands on the earliest-freed/spare banks
                with (
                    tc.tile_pool(name="rs_ps", bufs=2, space="PSUM") as rs_ps_pool,
                    tc.tile_pool(name="ot_ps", bufs=2, space="PSUM") as ot_ps_pool,
                    tc.tile_pool(name="st_ps", bufs=2, space="PSUM") as st_ps_pool,
                ):
                    PD = 5  # per-head pipeline depth
                    outT = outT_pool.tile([128, HPC * S], BF16)
                    outT_by_b[b] = outT
                    for qb in range(SB):
                        o_ps, r_ps = {}, {}
                        for h in range(HPC):
                            o_ps[h] = ot_ps_pool.tile(
                                [128, 512], F32, tag="ot", name=f"ot{h}"
                            )
                            r_ps[h] = rs_ps_pool.tile(
                                [128, 512], F32, tag="rs", name=f"rs{h}"
                            )
                        nj = 4 * qb + 4
                        pd = 3 if (b == B - 1 and qb == SB - 1) else PD
                        dfb = 24 if b < B - 1 else 0
                        pts = {}
                        s2s = {}
                        s4s = {}
                        diag = {}
                        for jj in range(nj + pd):
                            emit_op(2 if jj < 2 else 1, defer_below=dfb)
                            if jj < nj:
                                j = jj
                                r = j - 4 * qb  # diagonal band index
                                qoff = 128 * r if r > 0 else 0
                                W = 512 - qoff
                                for h in range(HPC):
                                    s_ps = st_ps_pool.tile([128, 512], F32)
                                    nc.tensor.matmul(
                                        s_ps[:, qoff:512],
                                        kT_sb[:, b * S + j * 128 : b * S + (j + 1) * 128],
                                        qk_sb[(h, b * SB + qb)][:, qoff:512],
                                        start=True,
                                        stop=True,
                                    )
                                    pt = pt_pool.tile([128, 512], BF16)
                                    nc.scalar.activation(
                                        pt[:, qoff:512],
                                        s_ps[:, qoff:512],
                                        AF.Exp,
                                        bias=expb_sb[:],
                                        scale=1.0,
                                    )
                                    if r >= 0:
                                        # zero where k > q inside the 128-wide
                                        # diagonal ramp
                                        nc.gpsimd.affine_select(
                                            out=pt[:, qoff : qoff + 128],
                                            in_=pt[:, qoff : qoff + 128],
                                            pattern=[[1, 128]],
                                            compare_op=ALU.is_ge,
                                            fill=0.0,
                                            base=0,
                                            channel_multiplier=-1,
                                        )
                                    pts[(h, j)] = (pt, qoff, W)
                                    # rowsum packing on DVE (all-bf16 = fast):
                                    padd = nc.vector.tensor_add
                                    pcopy = nc.vector.tensor_copy
                                    if j < 4 * qb:
                                        if j % 2 == 1:
                                            s2 = s2_pool.tile([128, 512], BF16, tag="s2")
                                            padd(s2[:], pts[(h, j - 1)][0][:], pt[:])
                                            s2s[(h, j // 2)] = s2
                                        if j % 4 == 3:
                                            s4 = s4_pool.tile([128, 512], BF16, tag="s4")
                                            padd(
                                                s4[:],
                                                s2s.pop((h, j // 2 - 1))[:],
                                                s2s.pop((h, j // 2))[:],
                                            )
                                            s4s[(h, j // 4)] = s4
                                    elif r == 1:
                                        pt0 = pts[(h, 4 * qb)][0]
                                        sa = s4_pool.tile([128, 512], BF16, tag="s4")
                                        pcopy(sa[:, 0:128], pt0[:, 0:128])
                                        padd(
                                            sa[:, 128:512],
                                            pt0[:, 128:512],
                                            pt[:, 128:512],
                                        )
                                        diag[(h, 0)] = sa
                                    elif r == 3:
                                        pt2 = pts[(h, 4 * qb + 2)][0]
                                        sb_ = s4_pool.tile([128, 512], BF16, tag="s4")
                                        pcopy(sb_[:, 256:384], pt2[:, 256:384])
                                        padd(
                                            sb_[:, 384:512],
                                            pt2[:, 384:512],
                                            pt[:, 384:512],
                                        )
                                        diag[(h, 1)] = sb_
                            if jj >= pd:
                                j2 = jj - pd
                                for h in range(HPC):
                                    pt2, qoff2, W2 = pts.pop((h, j2))
                                    if j2 < 4 * qb:
                                        if j2 % 4 == 3:
                                            s4c = s4s.pop((h, j2 // 4))
                                            nc.tensor.matmul(
                                                r_ps[h][:],
                                                ones_sb[:],
                                                s4c[:],
                                                start=(j2 == 3),
                                                stop=False,
                                                skip_group_check=True,
                                            )
                                    elif j2 == 4 * qb + 1:
                                        nc.tensor.matmul(
                                            r_ps[h][:],
                                            ones_sb[:],
                                            diag[(h, 0)][:],
                                            start=(qb == 0),
                                            stop=False,
                                            skip_group_check=True,
                                        )
                                    elif j2 == 4 * qb + 3:
                                        nc.tensor.matmul(
                                            r_ps[h][:, 256:512],
                                            ones_sb[:],
                                            diag[(h, 1)][:, 256:512],
                                            start=False,
                                            stop=True,
                                            skip_group_check=True,
                                        )
                                    nc.tensor.matmul(
                                        o_ps[h][:, qoff2:512],
                                        vall_sb[:, b * S + j2 * 128 : b * S + (j2 + 1) * 128],
                                        pt2[:, qoff2:512],
                                        start=(j2 == 0),
                                        stop=(j2 == nj - 1),
                                        skip_group_check=True,
                                    )
                            emit_op(1, defer_below=dfb)
                        for h in range(HPC):
                            rr = rr_sb_pool.tile([128, 512], F32, tag="rr")
                            nc.vector.reciprocal(rr[:], r_ps[h][:])
                            nc.vector.tensor_mul(
                                outT[:, h * S + qb * 512 : h * S + (qb + 1) * 512],
                                o_ps[h][:],
                                rr[:],
                            )
                        for stq in range(4):
                            for nb in range(4):
                                pending.append((b, outT, qb * 4 + stq, nb))
            # final drain: attention pools are closed, so spread the o_proj
            # psums over 6 banks to decouple the matmul stream from the
            # copy+DMA latency chain
            with tc.tile_pool(name="drain_ps", bufs=6, space="PSUM") as drain_pool:
                emit_op(len(pending), split=True, pool=drain_pool)
    nc.compile()
    return nc


_GRAPH = None


def _rope_tables():
    inv_freq = 1.0 / (10000.0 ** (np.arange(0, D, 2, dtype=np.float32) / D))
    t = np.arange(S, dtype=np.float32)
    freqs = np.outer(t, inv_freq)
    emb = np.concatenate([freqs, freqs], axis=-1)  # (S, D)
    cosT = np.ascontiguousarray(np.cos(emb).T.astype(np.float32))
    sinT = np.ascontiguousarray(np.sin(emb).T.astype(np.float32))
    sinadjT = sinT.copy()
    sinadjT[0:64, :] *= -1.0  # fold rotate_half's sign into the table
    return cosT, sinadjT


def kernel(x, wq, wk, wv, wo):
    global _GRAPH, LAST_EXEC_TIME_NS, LAST_RESULTS
    import ml_dtypes

    bf16 = ml_dtypes.bfloat16
    x = np.asarray(x, dtype=np.float32)
    wq = np.asarray(wq, dtype=np.float32)
    wk = np.asarray(wk, dtype=np.float32)
    wv = np.asarray(wv, dtype=np.float32)
    wo = np.asarray(wo, dtype=np.float32)

    xT = np.ascontiguousarray(x.reshape(R, H).T)
    # xTr[rb*128+p, hc*512+c] = xT[hc*128+p, rb*512+c]
    xTr = np.ascontiguousarray(
        xT.reshape(NB_HC, 128, NB_RB, 512).transpose(2, 1, 0, 3).reshape(
            NB_RB * 128, NB_HC * 512
        )
    ).astype(bf16)
    cosT, sinadjT = _rope_tables()
    cosT = cosT.astype(bf16)
    sinadjT = sinadjT.astype(bf16)
    scale = np.float32(1.0 / np.sqrt(D))

    in_maps = []
    for c in range(NCORES):
        kv = c // HPC
        wq_c = wq[:, c * HPC * D : (c + 1) * HPC * D] * scale
        wk_c = wk[:, kv * D : (kv + 1) * D]
        wv_c = wv[:, kv * D : (kv + 1) * D]
        wqkv_c = np.concatenate([wq_c, wk_c, wv_c], axis=1, dtype=np.float32)
        # wqkvr[p, hc*512+c] = wqkv_c[hc*128+p, c]
        wqkvr = np.ascontiguousarray(
            wqkv_c.reshape(NB_HC, 128, QKV_W).transpose(1, 0, 2).reshape(
                128, NB_HC * 512
            )
        ).astype(bf16)
        wo_c = wo[c * HPC * D : (c + 1) * HPC * D, :]
        wor = np.ascontiguousarray(
            wo_c.reshape(HPC, 128, H).transpose(1, 0, 2).reshape(128, HPC * H)
        ).astype(bf16)
        # kT dedup: each core sees its OWN batch first (even cores batch 0,
        # odd cores batch 1) and ropes kT only for it; the pair AllGather +
        # exact reconstruction supplies the partner half. The output rows
        # come back local-batch-first and are unpermuted below.
        if c % 2 == 0:
            xTr_c = xTr
        else:
            xTr_c = np.ascontiguousarray(
                np.concatenate([xTr[SB * 128 :], xTr[: SB * 128]], axis=0)
            )
        in_maps.append(
            {
                "xTr": xTr_c,
                "wqkvr": wqkvr,
                "wor": wor,
                "cosT": cosT,
                "sinadjT": sinadjT,
            }
        )

    if _GRAPH is None:
        _GRAPH = build_graph()

    # NTFF tracing is unavailable on axon clients without antenv.axon_hooks;
    # make sure an inherited BASS_TRACE can't break execution.
    os.environ["BASS_NEVER_TRACE"] = "1"
    res = None
    for attempt in range(3):
        try:
            res = run_bass_kernel_spmd(
                _GRAPH, in_maps, core_ids=list(range(NCORES))
            )
            break
        except Exception:
            # transient axon-terminal failures (mesh desync / LoadExecutable)
            # usually clear on retry
            if attempt == 2:
                raise
            time.sleep(5.0)
    LAST_EXEC_TIME_NS = res.exec_time_ns
    LAST_RESULTS = res
    acc = np.zeros((R, H), dtype=np.float32)
    for c in range(NCORES):
        part = np.asarray(res.results[c]["out"], dtype=np.float32)
        if c % 2 == 1:  # odd cores computed batch 1 in their first half
            part = np.concatenate([part[S:], part[:S]], axis=0)
        acc += part
    return acc.reshape(B, S, H)
